# revision 1
# baseline (speedup 1.0000x reference)
"""Trainium2 Bass kernel for nn_Attention_6820408066818 (gnn message passing).

Math (reference):
  local_pair[b,i,j,:] = lf[b,i,:] + lf[b,j,:]
  att = relu(local_pair @ Wa + bf @ Wbin + b_bin)          # [B,N,N,H]
  score = sigmoid(att @ w_att + b_att)                     # [B,N,N,1]
  gf[b,i,:] = sum_j score[b,i,j] * lf[b,j,:]               # [B,N,H]
  out1[e] = local_pair[be,ie,je]   out2[e] = gf[be,ie] + gf[be,je]

Key identity: local_pair @ Wa = P[i] + P[j] with P = lf @ Wa, so the big
[B,N,N,H] tensor is never materialized.  Per core (4 batches), everything is
computed in [H=128 partitions, (j,i) columns] layout; "pre" is produced by a
single K=122 matmul per 500-column chunk whose stationary operand packs, per
chunk c (j in [5c,5c+5)):
    K rows  0- 95 : P[i] rows (i<96)        <- identity(i) rhs rows
    K rows 96-100 : P[5c+r] rows            <- j-indicator rhs rows
    K rows 101-104: P[i] rows (i>=96)       <- identity(i) rhs rows
    K rows 105-120: Wbin                    <- bf^T rhs rows (c contraction)
    K row  121    : b_binary                <- all-ones rhs row
relu: PSUM->SBUF alternating ACT/DVE; score: K=128 matmul against a padded
w_att so chunk c lands on PSUM partition c; gf: att^T[j,i] @ lf[b] (K=j);
sparse gather via dma_gather on device-local rows.

Sharding: data-parallel over batch, 4 batches per core, 8 cores.
"""

import os
import sys

import numpy as np

sys.path.insert(0, "/opt/trn_rl_repo")

B, N, H, BIN, E = 32, 100, 128, 16, 20000
NCORES = 8
NB = B // NCORES          # batches per core
CJ = 5                    # j's per chunk
CC = CJ * N               # 500 columns per chunk
NCHUNK = N // CJ          # 20 chunks per batch
HALFC = NCHUNK // 2       # chunks per stitched tile
HCOLS = HALFC * CC        # 5000
GROUP = 2                 # chunks per relu group

# K-row layout
IND0 = N                  # j-indicator rows at [100, 105)
WB0 = N + CJ              # Wbin rows at [105, 121)
ONESR = WB0 + BIN         # 121: all-ones rhs row <-> b_binary lhsT row
K_TOT = ONESR + 1         # 122

_cache = {}


def _build_statics():
    """Stitched-rhs static image [K_TOT, CC]; bf rows [WB0,ONESR) left 0."""
    st = np.zeros((K_TOT, CC), dtype=np.float32)
    for jj in range(CJ):
        st[:N, jj * N:(jj + 1) * N] = np.eye(N, dtype=np.float32)
        st[IND0 + jj, jj * N:(jj + 1) * N] = 1.0
    st[ONESR, :] = 1.0
    return st


def _build_watt_pad(W_att):
    # w_att at column H of a [H, 2H] strip: window [H-c, 2H-c) has w_att at
    # relative column c only, so chunk c's score lands on PSUM partition c.
    wp = np.zeros((H, 2 * H), dtype=np.float32)
    wp[:, H] = W_att[:, 0]
    return wp



def _rep_mid(ap, n):
    """[p, h] -> [p, n, h] with the middle dim broadcast (step 0)."""
    p, h = ap.shape
    return ap.rearrange("k (u h) -> k u h", u=1).broadcast_to([p, n, h])

def _build_program():
    import concourse.mybir as mybir
    import concourse.tile as tile
    from concourse import bacc
    from contextlib import ExitStack

    f32 = mybir.dt.float32
    f32r = mybir.dt.float32r
    i16 = mybir.dt.int16

    GPAD = _cache["GPAD"]
    NG = GPAD // 128
    LW = NCHUNK * H           # big-lhsT width: 20 slices of 128

    nc = bacc.Bacc(
        "TRN2",
        target_bir_lowering=False,
        debug=False,
        enable_asserts=False,
        num_devices=NCORES,
    )

    # ---- DRAM I/O ----
    bf_d = nc.dram_tensor("bf_t", [NB, BIN, N * N], f32r, kind="ExternalInput").ap()
    lf_d = nc.dram_tensor("lf", [NB * N, H], f32r, kind="ExternalInput").ap()
    # blob packs wa | lf_t | lf_tq | watt | batt as [128, 1185]
    blob_d = nc.dram_tensor("blob", [H, 2 * NB * N + H + 2 * H + 1], f32r,
                            kind="ExternalInput").ap()
    wbp_d = nc.dram_tensor("wbp", [BIN + 1, H], f32r, kind="ExternalInput").ap()
    lfj_d = nc.dram_tensor("lfj", [NCHUNK, CJ * NB * H], f32r,
                           kind="ExternalInput").ap()
    stat_d = nc.dram_tensor("statics", [K_TOT, CC], f32r, kind="ExternalInput").ap()
    r1_d = nc.dram_tensor("r1", [128, GPAD // 16], i16, kind="ExternalInput").ap()
    r2_d = nc.dram_tensor("r2", [128, GPAD // 16], i16, kind="ExternalInput").ap()
    lp_d = nc.dram_tensor("lp_out", [GPAD, H], f32, kind="ExternalOutput").ap()
    gp_d = nc.dram_tensor("gp_out", [GPAD, H], f32, kind="ExternalOutput").ap()

    with tile.TileContext(nc) as tc, ExitStack() as ctx:
        const = ctx.enter_context(tc.tile_pool(name="const", bufs=1))
        stitched_p = ctx.enter_context(tc.tile_pool(name="stitched", bufs=1))
        big_p = ctx.enter_context(tc.tile_pool(name="biglhsT", bufs=1))
        perb_p = ctx.enter_context(tc.tile_pool(name="perb", bufs=3))
        r_p = ctx.enter_context(tc.tile_pool(name="relu", bufs=6))
        sig_p = ctx.enter_context(tc.tile_pool(name="sig", bufs=3))
        gath_p = ctx.enter_context(tc.tile_pool(name="gath", bufs=1))
        pre_psum = ctx.enter_context(tc.tile_pool(name="pre_ps", bufs=2, space="PSUM"))
        score_psum = ctx.enter_context(tc.tile_pool(name="sc_ps", bufs=2, space="PSUM"))
        misc_psum = ctx.enter_context(tc.tile_pool(name="mi_ps", bufs=1, space="PSUM"))
        dram_p = ctx.enter_context(tc.tile_pool(name="dram", bufs=1, space="DRAM"))

        # ---- one-time loads, ordered so batch 0's critical path starts
        # immediately (scheduler priority follows program order) ----
        blob_s = const.tile([H, 2 * NB * N + H + 2 * H + 1], f32r)
        nc.sync.dma_start(blob_s[:, 0:H + NB * N], blob_d[:, 0:H + NB * N])
        nc.scalar.dma_start(blob_s[:, H + NB * N:], blob_d[:, H + NB * N:])
        wa_s = blob_s[:, 0:H]
        lft_s = blob_s[:, H:H + NB * N]
        lftq_s = blob_s[:, H + NB * N:H + 2 * NB * N]
        watt_s = blob_s[:, H + 2 * NB * N:H + 2 * NB * N + 2 * H]
        batt_s = blob_s[:, H + 2 * NB * N + 2 * H:].bitcast(f32)

        # stitched rhs tiles: statics DMA'd into the first 500 columns then
        # replicated in-SBUF by the (otherwise idle) gpsimd engine; the bf
        # rows [WB0, ONESR) are rewritten per half-batch (3-way rotation)
        NST = 3
        st_tiles = []
        for t in range(NST):
            s = stitched_p.tile([K_TOT, HCOLS], f32r, tag=f"st{t}")
            st_tiles.append(s)

        def load_statics(t):
            eng = nc.sync if t % 2 == 0 else nc.scalar
            eng.dma_start(st_tiles[t][:, 0:CC], stat_d[:])
            nc.vector.tensor_copy(
                out=st_tiles[t][:, CC:HCOLS],
                in_=_rep_mid(st_tiles[t][:, 0:CC], HALFC - 1))

        def load_bf(b, h):
            eng = nc.sync if h == 0 else nc.scalar
            eng.dma_start(st_tiles[(2 * b + h) % NST][WB0:ONESR, :],
                          bf_d[b, :, h * HCOLS:(h + 1) * HCOLS])

        load_statics(0)
        nc.sync.dma_start(st_tiles[0][WB0:ONESR, 0:CC], bf_d[0, :, 0:CC])
        nc.sync.dma_start(st_tiles[0][WB0:ONESR, CC:HCOLS], bf_d[0, :, CC:HCOLS])

        # persistent big-lhsT tiles (Wb/bias rows written once, P rows per b)
        big_tiles = []
        for t in range(2):
            g = big_p.tile([K_TOT, LW], f32r, tag=f"big{t}")
            big_tiles.append(g)
        nc.sync.dma_start(big_tiles[0][WB0:K_TOT, :], _rep_mid(wbp_d[:], NCHUNK))

        load_statics(1)
        load_bf(0, 1)

        lfj_s = const.tile([NCHUNK, CJ * NB * H], f32r)

        gf_dram = dram_p.tile([NB * N, H], f32)
        GH = GPAD // 2
        NGH = GH // 128
        IW = GPAD // 16 // 2

        for b in range(NB):
            big = big_tiles[b % 2]

            # P and permuted-P matmuls share one PSUM bank (prep tile)
            prep_ps = misc_psum.tile([N, 2 * H], f32, tag="prep")
            p_ps = prep_ps[:, 0:H]
            nc.tensor.matmul(p_ps, lft_s[:, b * N:(b + 1) * N], wa_s,
                             start=True, stop=True)
            # identity-selected P rows: copy once, then replicate along slices
            nc.vector.tensor_copy(out=big[0:N, 0:H], in_=p_ps)
            nc.vector.tensor_copy(
                out=big[0:N, H:LW],
                in_=_rep_mid(big[0:N, 0:H], NCHUNK - 1))

            # indicator rows: second P matmul with permuted output rows
            # (partition q = r*NCHUNK + c holds P[5c+r]); row r of pq_s
            # flattens to big[IND0+r, :] = P[5c+r] per 128-col slice c.
            pq_ps = prep_ps[:, H:2 * H]
            nc.tensor.matmul(pq_ps, lftq_s[:, b * N:(b + 1) * N], wa_s,
                             start=True, stop=True)
            pq_s = perb_p.tile([N, H], f32r, tag="pq")
            nc.vector.tensor_copy(out=pq_s[:], in_=pq_ps)
            for r in range(CJ):
                eng = nc.sync if r % 2 == 0 else nc.scalar
                eng.dma_start(big[IND0 + r:IND0 + r + 1, :],
                              pq_s[r * NCHUNK:(r + 1) * NCHUNK, :])

            # load bf halves into stitched tiles (b0's were loaded early)
            if b > 0:
                load_bf(b, 0)
                load_bf(b, 1)

            sc_ps = score_psum.tile([H, CC], f32, tag="sc")

            # score matmuls for group g are emitted after group g+1's pre
            # matmuls so PE fills the relu latency instead of stalling
            pend = None

            def emit_scores(rs, gix):
                for u in range(GROUP):
                    c = gix * GROUP + u
                    nc.tensor.matmul(sc_ps[:], watt_s[:, H - c:2 * H - c],
                                     rs[:, u * CC:(u + 1) * CC],
                                     start=(c == 0), stop=(c == NCHUNK - 1))

            for gidx in range(NCHUNK // GROUP):
                pre_ps = pre_psum.tile([H, GROUP * 512], f32, tag="pre")
                r_s = r_p.tile([H, GROUP * CC], f32r, tag="r")
                for u in range(GROUP):
                    c = gidx * GROUP + u
                    sthalf = st_tiles[(2 * b + c // HALFC) % NST]
                    col0 = (c % HALFC) * CC
                    nc.tensor.matmul(pre_ps[:, u * 512:u * 512 + CC],
                                     big[:, c * H:(c + 1) * H],
                                     sthalf[:, col0:col0 + CC],
                                     start=True, stop=True)
                if pend is not None:
                    emit_scores(*pend)
                # relu PSUM -> SBUF: both engines in parallel (latency)
                nc.scalar.activation(r_s[:, 0:CC], pre_ps[:, 0:CC],
                                     mybir.ActivationFunctionType.Relu)
                nc.vector.tensor_scalar_max(r_s[:, CC:2 * CC],
                                            pre_ps[:, 512:512 + CC], 0.0)
                pend = (r_s, gidx)
            emit_scores(*pend)

            if b == 0:
                # deferred, non-critical loads: emitted after batch 0's
                # chunk loop so they don't compete with its critical DMAs
                nc.scalar.dma_start(lfj_s[:], lfj_d[:])
                load_statics(2)
                nc.scalar.dma_start(big_tiles[1][WB0:K_TOT, :],
                                    _rep_mid(wbp_d[:], NCHUNK))

            # sigmoid(score + b_att) on rows 0..19 -> [20(c), (jj, i)]
            sig_s = sig_p.tile([NCHUNK, CC], f32r, tag="sig")
            nc.scalar.activation(sig_s[:], sc_ps[0:NCHUNK, :],
                                 mybir.ActivationFunctionType.Sigmoid,
                                 bias=batt_s[0:NCHUNK, :])

            # gf[b] = sum_jj att_jj^T @ lf rows {j = 5c+jj}: 5 accumulating
            # K=20 matmuls straight out of sig_s (no att reshape needed)
            gf_ps = misc_psum.tile([N, H], f32, tag="gf")
            for jj in range(CJ):
                rhs = lfj_s[:, (jj * NB + b) * H:(jj * NB + b + 1) * H]
                nc.tensor.matmul(gf_ps[:], sig_s[:, jj * N:(jj + 1) * N], rhs,
                                 start=(jj == 0), stop=(jj == CJ - 1))
            gf_s = perb_p.tile([N, H], f32, tag="gfs")
            nc.scalar.copy(gf_s[:], gf_ps[:])
            nc.scalar.dma_start(gf_dram[b * N:(b + 1) * N, :], gf_s[:])

            if b == 1:
                # lf-row gathers: independent of the main compute; emitted
                # after this batch's bf loads so those keep DMA priority
                r1_s = gath_p.tile([128, GPAD // 16], i16, tag="r1")
                nc.sync.dma_start(r1_s[:], r1_d[:])
                r2_s = gath_p.tile([128, GPAD // 16], i16, tag="r2")
                nc.scalar.dma_start(r2_s[:], r2_d[:])
            if b in (1, 2):
                hh = b - 1
                ga = gath_p.tile([128, NGH, H], f32, tag="lpa")
                gb = gath_p.tile([128, NGH, H], f32, tag="lpb")
                nc.gpsimd.dma_gather(ga[:], lf_d[:].bitcast(f32),
                                     r1_s[:, hh * IW:(hh + 1) * IW],
                                     GH, GH, H, single_packet=False)
                nc.gpsimd.dma_gather(gb[:], lf_d[:].bitcast(f32),
                                     r2_s[:, hh * IW:(hh + 1) * IW],
                                     GH, GH, H, single_packet=False)
                gsum = gath_p.tile([128, NGH, H], f32, tag="lps")
                nc.gpsimd.tensor_tensor(out=gsum[:], in0=ga[:], in1=gb[:],
                                        op=mybir.AluOpType.add)
                nc.sync.dma_start(
                    lp_d[hh * GH:(hh + 1) * GH, :].rearrange(
                        "(g p) e -> p g e", p=128), gsum[:])

        # gf-row gathers (after all gf writes)
        for hh in range(2):
            ga = gath_p.tile([128, NGH, H], f32, tag="gpa")
            gb = gath_p.tile([128, NGH, H], f32, tag="gpb")
            nc.gpsimd.dma_gather(ga[:], gf_dram[:],
                                 r1_s[:, hh * IW:(hh + 1) * IW], GH, GH, H,
                                 single_packet=False)
            nc.gpsimd.dma_gather(gb[:], gf_dram[:],
                                 r2_s[:, hh * IW:(hh + 1) * IW], GH, GH, H,
                                 single_packet=False)
            gsum = gath_p.tile([128, NGH, H], f32, tag="gps")
            nc.gpsimd.tensor_tensor(out=gsum[:], in0=ga[:], in1=gb[:],
                                    op=mybir.AluOpType.add)
            nc.scalar.dma_start(
                gp_d[hh * GH:(hh + 1) * GH, :].rearrange(
                    "(g p) e -> p g e", p=128), gsum[:])

    nc.compile()
    return nc


def _host_prep(local_feats, binary_feats, sparse_idx, W_apair, W_binary,
               b_binary, W_att, b_att):
    """Shard + lay out inputs per core; returns (in_maps, scatter info)."""
    lf = np.asarray(local_feats, dtype=np.float32)
    bf = np.asarray(binary_feats, dtype=np.float32)
    si = np.asarray(sparse_idx)

    b_idx, i_idx, j_idx = si[:, 0], si[:, 1], si[:, 2]
    core = (b_idx // NB).astype(np.int64)
    orders, counts = [], []
    for k in range(NCORES):
        rows = np.nonzero(core == k)[0]
        orders.append(rows)
        counts.append(len(rows))
    gmax = max(counts)
    GPAD = max(256, ((gmax + 255) // 256) * 256)
    _cache["GPAD"] = GPAD

    statics = _build_statics()
    watt = _build_watt_pad(np.asarray(W_att, dtype=np.float32))
    wbp = np.concatenate([np.asarray(W_binary, dtype=np.float32),
                          np.asarray(b_binary, dtype=np.float32).reshape(1, H)])
    batt = np.full((H, 1), np.float32(np.asarray(b_att).reshape(-1)[0]),
                   dtype=np.float32)
    wa = np.ascontiguousarray(np.asarray(W_apair, dtype=np.float32))

    in_maps = []
    for k in range(NCORES):
        b0 = k * NB
        lf_k = np.ascontiguousarray(lf[b0:b0 + NB].reshape(NB * N, H))
        lft_k = np.ascontiguousarray(lf_k.T)
        # column q of batch b = lf[b, CJ*(q%NCHUNK) + q//NCHUNK]
        qperm = (CJ * (np.arange(N) % NCHUNK) + np.arange(N) // NCHUNK)
        lftq_k = np.ascontiguousarray(
            lf[b0:b0 + NB][:, qperm].reshape(NB * N, H).T)
        # lfj tile: row c, col-block (g, b) holds lf[b, 5c+g]
        lfj_k = np.zeros((NCHUNK, CJ * NB * H), dtype=np.float32)
        for g in range(CJ):
            for c in range(NCHUNK):
                lfj_k[c, g * NB * H:(g + 1) * NB * H] = \
                    lf[b0:b0 + NB, CJ * c + g, :].reshape(-1)
        # [b, i, j, c] -> [b, c, (j, i)]
        bft_k = np.ascontiguousarray(
            bf[b0:b0 + NB].transpose(0, 3, 2, 1).reshape(NB, BIN, N * N))
        rows = orders[k]
        r1 = ((b_idx[rows] - b0) * N + i_idx[rows]).astype(np.int16)
        r2 = ((b_idx[rows] - b0) * N + j_idx[rows]).astype(np.int16)
        r1 = np.concatenate([r1, np.zeros(GPAD - len(r1), np.int16)])
        r2 = np.concatenate([r2, np.zeros(GPAD - len(r2), np.int16)])
        blob = np.concatenate(
            [wa, lft_k, lftq_k, watt, batt.astype(np.float32)], axis=1)
        in_maps.append({
            "bf_t": bft_k, "lf": lf_k, "blob": blob, "lfj": lfj_k,
            "wbp": wbp, "statics": statics,
            "r1": _wrap_idx(r1), "r2": _wrap_idx(r2),
        })
    return in_maps, orders, counts, GPAD


def _wrap_idx(idx):
    # int16 indices wrapped in 16 partitions (idx[k] at [k % 16, k // 16]),
    # replicated to all 8 gpsimd cores' partition groups.
    w = idx.reshape(-1, 16).T
    return np.ascontiguousarray(np.tile(w, (8, 1)))


def kernel(local_feats, binary_feats, sparse_idx, W_apair, W_binary,
           b_binary, W_att, b_att):
    in_maps, orders, counts, GPAD = _host_prep(
        local_feats, binary_feats, sparse_idx, W_apair, W_binary,
        b_binary, W_att, b_att)

    key = ("prog", GPAD)
    if key not in _cache:
        _cache[key] = _build_program()
    nc = _cache[key]

    from concourse.bass_utils import run_bass_kernel_spmd
    trace = os.environ.get("KERNEL_TRACE", "0") == "1"
    res = run_bass_kernel_spmd(nc, in_maps, core_ids=list(range(NCORES)),
                               trace=trace)
    if trace and res.exec_time_ns is not None:
        print(f"HW exec time: {res.exec_time_ns} ns")
        print(f"mean exec time: {res.mean_exec_time_ns} ns")
        if res.instructions_and_trace is not None:
            print(f"trace: {res.instructions_and_trace[1]}")

    lp_full = np.empty((E, H), dtype=np.float32)
    gp_full = np.empty((E, H), dtype=np.float32)
    for k in range(NCORES):
        out = res.results[k]
        lp_full[orders[k]] = out["lp_out"][:counts[k]]
        gp_full[orders[k]] = out["gp_out"][:counts[k]]
    return (lp_full, gp_full)



# revision 8
# speedup vs baseline: 1.1623x; 1.1623x over previous
"""Trainium2 Bass kernel for nn_Attention_6820408066818 (gnn message passing).

Math (reference):
  local_pair[b,i,j,:] = lf[b,i,:] + lf[b,j,:]
  att = relu(local_pair @ Wa + bf @ Wbin + b_bin)          # [B,N,N,H]
  score = sigmoid(att @ w_att + b_att)                     # [B,N,N,1]
  gf[b,i,:] = sum_j score[b,i,j] * lf[b,j,:]               # [B,N,H]
  out1[e] = local_pair[be,ie,je]   out2[e] = gf[be,ie] + gf[be,je]

Key identity: local_pair @ Wa = P[i] + P[j] with P = lf @ Wa, so the big
[B,N,N,H] tensor is never materialized.  Per core (4 batches), everything is
computed in [H=128 partitions, (j,i) columns] layout; "pre" is produced by a
single K=122 matmul per 500-column chunk whose stationary operand packs, per
chunk c (j in [5c,5c+5)):
    K rows  0- 99 : P[i] rows               <- identity(i) rhs rows
    K rows 100-104: P[5c+r] rows            <- j-indicator rhs rows
    K row  105    : b_binary                <- all-ones rhs row
    K rows 106-121: Wbin                    <- bf^T rhs rows (c contraction)
The whole matmul pipeline runs in bf16 (tolerance is 2e-2); accumulation
stays fp32 in PSUM.  score: K=128 matmul against a padded w_att so chunk c
lands on PSUM partition c; gf: one K=100 matmul from a (jj,c)-ordered
sigmoid tile; sparse gather per batch from a combined [lf|gf] bf16 row
table so each batch's edge chains overlap the next batch's compute.

Sharding: data-parallel over batch, 4 batches per core, 8 cores.
"""

import os
import sys

import numpy as np
import ml_dtypes

sys.path.insert(0, "/opt/trn_rl_repo")

B, N, H, BIN, E = 32, 100, 128, 16, 20000
NCORES = 8
NB = B // NCORES          # batches per core
CJ = 5                    # j's per chunk
CC = CJ * N               # 500 columns per chunk
NCHUNK = N // CJ          # 20 chunks per batch
HALFC = NCHUNK // 2       # chunks per stitched tile
HCOLS = HALFC * CC        # 5000
GROUP = 2                 # chunks per relu group

# K-row layout
IND0 = N                  # j-indicator rows at [100, 105)
ONESR = N + CJ            # 105: all-ones rhs row <-> b_binary lhsT row
WB0 = ONESR + 1           # Wbin rows at [106, 122)
K_TOT = WB0 + BIN         # 122
NSTAT = ONESR + 1         # 106 static rhs rows (identity+ind+ones)

BF16 = ml_dtypes.bfloat16

_cache = {}


def _build_statics():
    """Static rhs rows [NSTAT, CC]; bf rows live at [WB0, K_TOT) below."""
    st = np.zeros((NSTAT, CC), dtype=np.float32)
    for jj in range(CJ):
        st[:N, jj * N:(jj + 1) * N] = np.eye(N, dtype=np.float32)
        st[IND0 + jj, jj * N:(jj + 1) * N] = 1.0
    st[ONESR, :] = 1.0
    return st.astype(BF16)


def _build_watt_pad(W_att):
    # w_att at column H of a [H, 2H] strip: window [H-c, 2H-c) has w_att at
    # relative column c only, so chunk c's score lands on PSUM partition c.
    wp = np.zeros((H, 2 * H), dtype=np.float32)
    wp[:, H] = W_att[:, 0]
    return wp.astype(BF16)


def _rep_mid(ap, n):
    """[p, h] -> [p, n, h] with the middle dim broadcast (step 0)."""
    p, h = ap.shape
    return ap.rearrange("k (u h) -> k u h", u=1).broadcast_to([p, n, h])


def _build_program():
    import concourse.mybir as mybir
    import concourse.tile as tile
    from concourse import bacc
    from contextlib import ExitStack

    f32 = mybir.dt.float32
    bf16 = mybir.dt.bfloat16
    i16 = mybir.dt.int16

    PB = _cache["PB"]         # padded edges per (core, batch)
    NGB = PB // 128           # gather column blocks per segment
    IWB = 2 * PB // 16        # idx cols per batch in the wrapped tile
    LW = NCHUNK * H           # big-lhsT width: 20 slices of 128
    BLOBW = H + 2 * NB * N + 2 * H   # wa | lft | lftq | watt

    nc = bacc.Bacc(
        "TRN2",
        target_bir_lowering=False,
        debug=False,
        enable_asserts=False,
        num_devices=NCORES,
    )

    # ---- DRAM I/O ----
    bf_d = nc.dram_tensor("bf_t", [NB, BIN, N * N], bf16, kind="ExternalInput").ap()
    blob_d = nc.dram_tensor("blob", [H, BLOBW], bf16, kind="ExternalInput").ap()
    batt_d = nc.dram_tensor("batt", [NCHUNK, 1], f32, kind="ExternalInput").ap()
    wbp_d = nc.dram_tensor("wbp", [1 + BIN, H], bf16, kind="ExternalInput").ap()
    lfbi_d = nc.dram_tensor("lfbi", [N, NB * H], bf16, kind="ExternalInput").ap()
    stat_d = nc.dram_tensor("statics", [NSTAT, CC], bf16, kind="ExternalInput").ap()
    tlf_d = nc.dram_tensor("tbl_lf", [NB * N, H], bf16, kind="ExternalInput").ap()
    ridx_d = nc.dram_tensor("ridx", [128, NB * IWB], i16, kind="ExternalInput").ap()
    out_d = nc.dram_tensor("out", [NB * PB, 2 * H], bf16, kind="ExternalOutput").ap()

    with tile.TileContext(nc) as tc, ExitStack() as ctx:
        const = ctx.enter_context(tc.tile_pool(name="const", bufs=1))
        stitched_p = ctx.enter_context(tc.tile_pool(name="stitched", bufs=1))
        big_p = ctx.enter_context(tc.tile_pool(name="biglhsT", bufs=1))
        perb_p = ctx.enter_context(tc.tile_pool(name="perb", bufs=2))
        r_p = ctx.enter_context(tc.tile_pool(name="relu", bufs=6))
        gath_p = ctx.enter_context(tc.tile_pool(name="gath", bufs=2))
        pre_psum = ctx.enter_context(tc.tile_pool(name="pre_ps", bufs=2, space="PSUM"))
        score_psum = ctx.enter_context(tc.tile_pool(name="sc_ps", bufs=2, space="PSUM"))
        misc_psum = ctx.enter_context(tc.tile_pool(name="mi_ps", bufs=1, space="PSUM"))
        dram_p = ctx.enter_context(tc.tile_pool(name="dram", bufs=1, space="DRAM"))

        # ---- one-time loads, ordered so batch 0's critical path starts
        # immediately (scheduler priority follows program order) ----
        blob_s = const.tile([H, BLOBW], bf16)
        nc.sync.dma_start(blob_s[:, 0:H + NB * N], blob_d[:, 0:H + NB * N])
        nc.sync.dma_start(blob_s[:, H + NB * N:], blob_d[:, H + NB * N:])
        wa_s = blob_s[:, 0:H]
        lft_s = blob_s[:, H:H + NB * N]
        lftq_s = blob_s[:, H + NB * N:H + 2 * NB * N]
        watt_s = blob_s[:, H + 2 * NB * N:]

        # stitched rhs tiles: static rows [0, NSTAT) + bf rows [WB0, K_TOT).
        # Statics are DMA'd into the first 500 columns, then replicated by
        # DVE/ACT; bf rows are disjoint so their DMAs never wait on the
        # replication.
        NST = 3
        st_tiles = []
        for t in range(NST):
            st_t = stitched_p.tile([K_TOT, HCOLS], bf16, tag=f"st{t}")
            st_tiles.append(st_t)

        def load_statics(t):
            nc.sync.dma_start(st_tiles[t][0:NSTAT, 0:CC], stat_d[:])

        def rep_statics(t):
            # replicate cols [CC, HCOLS) of the static rows, split ACT/DVE
            s = st_tiles[t]
            nc.scalar.copy(
                s[0:NSTAT, CC:4 * CC],
                _rep_mid(s[0:NSTAT, 0:CC], 3))
            nc.vector.tensor_copy(
                out=s[0:NSTAT, 4 * CC:HCOLS],
                in_=_rep_mid(s[0:NSTAT, 0:CC], HALFC - 4))

        def load_bf(b, h):
            nc.sync.dma_start(st_tiles[(2 * b + h) % NST][WB0:K_TOT, :],
                              bf_d[b, :, h * HCOLS:(h + 1) * HCOLS])

        batt_s = const.tile([NCHUNK, 1], f32)
        nc.sync.dma_start(batt_s[:], batt_d[:])

        load_statics(0)
        load_bf(0, 0)
        load_bf(0, 1)

        # combined [lf | gf] row table in DRAM; left half loaded once, right
        # half written per batch as gf completes.
        tbl = dram_p.tile([NB * N, 2 * H], bf16)
        nc.sync.dma_start(tbl[:, 0:H], tlf_d[:])

        # persistent big-lhsT tiles (Wb/bias rows written once, P rows per b)
        big_tiles = []
        for t in range(2):
            big_t = big_p.tile([K_TOT, LW], bf16, tag=f"big{t}")
            big_tiles.append(big_t)
        nc.sync.dma_start(big_tiles[0][ONESR:K_TOT, :], _rep_mid(wbp_d[:], NCHUNK))
        nc.sync.dma_start(big_tiles[1][ONESR:K_TOT, :], _rep_mid(wbp_d[:], NCHUNK))

        ridx_s = const.tile([128, NB * IWB], i16)
        nc.sync.dma_start(ridx_s[:], ridx_d[:])
        lfbi_s = const.tile([N, NB * H], bf16)
        nc.sync.dma_start(lfbi_s[:], lfbi_d[:])

        load_statics(1)

        def p_setup(b):
            """P/Pq matmuls + big-tile row fills for batch b."""
            big = big_tiles[b % 2]
            prep_ps = misc_psum.tile([N, 2 * H], f32, tag="prep")
            p_ps = prep_ps[:, 0:H]
            nc.tensor.matmul(p_ps, lft_s[:, b * N:(b + 1) * N], wa_s,
                             start=True, stop=True)
            nc.vector.tensor_copy(out=big[0:N, 0:H], in_=p_ps)
            # replicate P rows along the 20 slices (gpsimd has the slack)
            nc.gpsimd.tensor_copy(
                out=big[0:N, H:LW],
                in_=_rep_mid(big[0:N, 0:H], NCHUNK - 1))
            # indicator rows: second P matmul with permuted output rows
            # (partition q = r*NCHUNK + c holds P[5c+r]); one DMA scatters
            # row-chunks of pq_s into big[IND0+r, c*H:(c+1)*H].
            pq_ps = prep_ps[:, H:2 * H]
            nc.tensor.matmul(pq_ps, lftq_s[:, b * N:(b + 1) * N], wa_s,
                             start=True, stop=True)
            pq_s = perb_p.tile([N, H], bf16, tag="pq")
            nc.vector.tensor_copy(out=pq_s[:], in_=pq_ps)
            nc.sync.dma_start(
                big[IND0:ONESR, :],
                pq_s[:].rearrange("(r c) h -> r (c h)", r=CJ))

        p_setup(0)
        rep_statics(0)

        for b in range(NB):
            big = big_tiles[b % 2]

            if b == 1:
                rep_statics(1)
            if b == 0:
                load_bf(1, 0)
                load_bf(1, 1)
                load_statics(2)

            sc_ps = score_psum.tile([H, CC], f32, tag="sc")

            # score matmuls for group g are emitted after group g+1's pre
            # matmuls so PE fills the relu latency instead of stalling
            pend = None

            def emit_scores(rs, gix):
                for u in range(GROUP):
                    c = gix * GROUP + u
                    nc.tensor.matmul(sc_ps[:], watt_s[:, H - c:2 * H - c],
                                     rs[:, u * CC:(u + 1) * CC],
                                     start=(c == 0), stop=(c == NCHUNK - 1))

            for gidx in range(NCHUNK // GROUP):
                pre_ps = pre_psum.tile([H, GROUP * 512], f32, tag="pre")
                r_s = r_p.tile([H, GROUP * CC], bf16, tag="r")
                for u in range(GROUP):
                    c = gidx * GROUP + u
                    sthalf = st_tiles[(2 * b + c // HALFC) % NST]
                    col0 = (c % HALFC) * CC
                    nc.tensor.matmul(pre_ps[:, u * 512:u * 512 + CC],
                                     big[:, c * H:(c + 1) * H],
                                     sthalf[:, col0:col0 + CC],
                                     start=True, stop=True)
                if pend is not None:
                    emit_scores(*pend)
                # relu PSUM -> SBUF: both engines in parallel (latency)
                nc.scalar.activation(r_s[:, 0:CC], pre_ps[:, 0:CC],
                                     mybir.ActivationFunctionType.Relu)
                nc.vector.tensor_scalar_max(r_s[:, CC:2 * CC],
                                            pre_ps[:, 512:512 + CC], 0.0)
                pend = (r_s, gidx)
                if gidx == 4:
                    if b < NB - 1:
                        p_setup(b + 1)
                    if b == 2:
                        rep_statics(2)
                if gidx == 6 and b < NB - 2:
                    load_bf(b + 2, 0)
                    load_bf(b + 2, 1)
            emit_scores(*pend)

            # sigmoid into (jj,c)-ordered rows: partition jj*20+c holds
            # score[j=5c+jj, :], matching lfbi's row order, so gf is one
            # K=100 matmul.
            sig2 = perb_p.tile([N, N], bf16, tag="sig")
            for jj in range(CJ):
                nc.scalar.activation(sig2[jj * NCHUNK:(jj + 1) * NCHUNK, :],
                                     sc_ps[0:NCHUNK, jj * N:(jj + 1) * N],
                                     mybir.ActivationFunctionType.Sigmoid,
                                     bias=batt_s[:])
            gf_ps = misc_psum.tile([N, H], f32, tag="gf")
            nc.tensor.matmul(gf_ps[:], sig2[:],
                             lfbi_s[:, b * H:(b + 1) * H],
                             start=True, stop=True)
            gf_s = perb_p.tile([N, H], bf16, tag="gfs")
            nc.scalar.copy(gf_s[:], gf_ps[:])
            nc.scalar.dma_start(tbl[b * N:(b + 1) * N, H:2 * H], gf_s[:])

            # gather lf|gf rows for this batch's edges; first PB idxs are
            # r1 (-> out blocks [0,NGB)), next PB are r2.
            gout = gath_p.tile([128, 2 * NGB, 2 * H], bf16, tag="gout")
            nc.gpsimd.dma_gather(gout[:], tbl[b * N:(b + 1) * N, :],
                                 ridx_s[:, b * IWB:(b + 1) * IWB],
                                 2 * PB, 2 * PB, 2 * H, single_packet=False)
            gsum = gath_p.tile([128, NGB, 2 * H], bf16, tag="gsum")
            nc.vector.tensor_tensor(out=gsum[:], in0=gout[:, 0:NGB, :],
                                    in1=gout[:, NGB:2 * NGB, :],
                                    op=mybir.AluOpType.add)
            nc.scalar.dma_start(
                out_d[b * PB:(b + 1) * PB, :].rearrange(
                    "(p g) e -> p (g e)", p=128), gsum[:])

    nc.compile()
    return nc


def _host_prep(local_feats, binary_feats, sparse_idx, W_apair, W_binary,
               b_binary, W_att, b_att):
    """Shard + lay out inputs per core; returns (in_maps, unshard info)."""
    lf = np.asarray(local_feats, dtype=np.float32)
    bf = np.asarray(binary_feats, dtype=np.float32)
    si = np.asarray(sparse_idx)

    b_idx, i_idx, j_idx = si[:, 0], si[:, 1], si[:, 2]
    core = (b_idx // NB).astype(np.int64)
    lbat = (b_idx % NB).astype(np.int64)
    seg_rows = [[np.nonzero((core == k) & (lbat == b))[0]
                 for b in range(NB)] for k in range(NCORES)]
    maxc = max(len(r) for segs in seg_rows for r in segs)
    PB = max(128, ((maxc + 127) // 128) * 128)
    NGB = PB // 128
    _cache["PB"] = PB
    _cache["GPAD"] = NB * PB

    statics = _build_statics()
    watt = _build_watt_pad(np.asarray(W_att, dtype=np.float32))
    wbp = np.concatenate([np.asarray(b_binary, dtype=np.float32).reshape(1, H),
                          np.asarray(W_binary, dtype=np.float32)]).astype(BF16)
    batt = np.full((NCHUNK, 1), np.float32(np.asarray(b_att).reshape(-1)[0]),
                   dtype=np.float32)
    wa = np.asarray(W_apair, dtype=np.float32).astype(BF16)

    in_maps = []
    for k in range(NCORES):
        b0 = k * NB
        lf_k = lf[b0:b0 + NB].reshape(NB * N, H)
        lft_k = lf_k.T.astype(BF16)
        # column q of batch b = lf[b, CJ*(q%NCHUNK) + q//NCHUNK]
        qperm = (CJ * (np.arange(N) % NCHUNK) + np.arange(N) // NCHUNK)
        lftq_k = lf[b0:b0 + NB][:, qperm].reshape(NB * N, H).T.astype(BF16)
        # lfbi row jj*20+c = lf[b, 5c+jj] (matches sig2's partition order)
        jperm = (CJ * (np.arange(N) % NCHUNK) + np.arange(N) // NCHUNK)
        lfbi_k = np.ascontiguousarray(
            lf[b0:b0 + NB][:, jperm].transpose(1, 0, 2).reshape(N, NB * H)
        ).astype(BF16)
        # [b, i, j, c] -> [b, c, (j, i)]
        bft_k = np.ascontiguousarray(
            bf[b0:b0 + NB].transpose(0, 3, 2, 1).reshape(NB, BIN, N * N)
        ).astype(BF16)
        # per-batch p-major gather index streams [r1 block | r2 block]
        ridx = np.zeros(NB * 2 * PB, dtype=np.int16)
        for b in range(NB):
            rows = seg_rows[k][b]
            r1 = np.zeros(PB, dtype=np.int16)
            r2 = np.zeros(PB, dtype=np.int16)
            r1[:len(rows)] = i_idx[rows].astype(np.int16)
            r2[:len(rows)] = j_idx[rows].astype(np.int16)
            # slot [p, g] <- idx[g*128+p]; out row p*NGB+g = seg position
            a1 = r1.reshape(128, NGB).T.reshape(-1)
            a2 = r2.reshape(128, NGB).T.reshape(-1)
            ridx[b * 2 * PB:b * 2 * PB + PB] = a1
            ridx[b * 2 * PB + PB:(b + 1) * 2 * PB] = a2
        blob = np.concatenate([wa, lft_k, lftq_k, watt], axis=1)
        in_maps.append({
            "bf_t": bft_k, "blob": blob, "batt": batt, "wbp": wbp,
            "lfbi": lfbi_k, "statics": statics,
            "tbl_lf": lf_k.astype(BF16),
            "ridx": _wrap_idx(ridx),
        })
    return in_maps, seg_rows, PB


def _wrap_idx(idx):
    # int16 indices wrapped in 16 partitions (idx[k] at [k % 16, k // 16]),
    # replicated to all 8 gpsimd cores' partition groups.
    w = idx.reshape(-1, 16).T
    return np.ascontiguousarray(np.tile(w, (8, 1)))


def kernel(local_feats, binary_feats, sparse_idx, W_apair, W_binary,
           b_binary, W_att, b_att):
    in_maps, seg_rows, PB = _host_prep(
        local_feats, binary_feats, sparse_idx, W_apair, W_binary,
        b_binary, W_att, b_att)

    key = ("prog", NB * PB)
    if key not in _cache:
        _cache[key] = _build_program()
    nc = _cache[key]

    from concourse.bass_utils import run_bass_kernel_spmd
    trace = os.environ.get("KERNEL_TRACE", "0") == "1"
    res = run_bass_kernel_spmd(nc, in_maps, core_ids=list(range(NCORES)),
                               trace=trace)
    if trace and res.exec_time_ns is not None:
        print(f"HW exec time: {res.exec_time_ns} ns")
        print(f"mean exec time: {res.mean_exec_time_ns} ns")
        if res.instructions_and_trace is not None:
            print(f"trace: {res.instructions_and_trace[1]}")

    lp_full = np.empty((E, H), dtype=np.float32)
    gp_full = np.empty((E, H), dtype=np.float32)
    for k in range(NCORES):
        out = np.asarray(res.results[k]["out"]).astype(np.float32)
        for b in range(NB):
            rows = seg_rows[k][b]
            seg = out[b * PB:b * PB + len(rows)]
            lp_full[rows] = seg[:, 0:H]
            gp_full[rows] = seg[:, H:2 * H]
    return (lp_full, gp_full)


# revision 13
# speedup vs baseline: 1.2121x; 1.0428x over previous
"""Trainium2 Bass kernel for nn_Attention_6820408066818 (gnn message passing).

Math (reference):
  local_pair[b,i,j,:] = lf[b,i,:] + lf[b,j,:]
  att = relu(local_pair @ Wa + bf @ Wbin + b_bin)          # [B,N,N,H]
  score = sigmoid(att @ w_att + b_att)                     # [B,N,N,1]
  gf[b,i,:] = sum_j score[b,i,j] * lf[b,j,:]               # [B,N,H]
  out1[e] = local_pair[be,ie,je]   out2[e] = gf[be,ie] + gf[be,je]

Key identity: local_pair @ Wa = P[i] + P[j] with P = lf @ Wa, so the big
[B,N,N,H] tensor is never materialized.  Per core (4 batches), everything is
computed in [H=128 partitions, (j,i) columns] layout; "pre" is produced by a
single K=122 matmul per 500-column chunk whose stationary operand packs, per
chunk c (j in [5c,5c+5)):
    K rows  0- 99 : P[i] rows               <- identity(i) rhs rows
    K rows 100-104: P[5c+r] rows            <- j-indicator rhs rows
    K row  105    : b_binary                <- all-ones rhs row
    K rows 106-121: Wbin                    <- bf^T rhs rows (c contraction)
The whole matmul pipeline runs in bf16 (tolerance is 2e-2); accumulation
stays fp32 in PSUM.  score: K=128 matmul against a padded w_att so chunk c
lands on PSUM partition c; gf: one K=100 matmul from a (jj,c)-ordered
sigmoid tile; sparse gather per batch from a combined [lf|gf] bf16 row
table so each batch's edge chains overlap the next batch's compute.

Sharding: data-parallel over batch, 4 batches per core, 8 cores.
"""

import os
import sys

import numpy as np
import ml_dtypes

sys.path.insert(0, "/opt/trn_rl_repo")

B, N, H, BIN, E = 32, 100, 128, 16, 20000
NCORES = 8
NB = B // NCORES          # batches per core
CJ = 5                    # j's per chunk
CC = CJ * N               # 500 columns per chunk
NCHUNK = N // CJ          # 20 chunks per batch
HALFC = NCHUNK // 2       # chunks per stitched tile
HCOLS = HALFC * CC        # 5000
GROUP = 2                 # chunks per relu group

# K-row layout
IND0 = N                  # j-indicator rows at [100, 105)
ONESR = N + CJ            # 105: all-ones rhs row <-> b_binary lhsT row
WB0 = ONESR + 1           # Wbin rows at [106, 122)
K_TOT = WB0 + BIN         # 122
NSTAT = ONESR + 1         # 106 static rhs rows (identity+ind+ones)

BF16 = ml_dtypes.bfloat16

_cache = {}


def _build_statics():
    """Static rhs rows [NSTAT, CC]; bf rows live at [WB0, K_TOT) below."""
    st = np.zeros((NSTAT, CC), dtype=np.float32)
    for jj in range(CJ):
        st[:N, jj * N:(jj + 1) * N] = np.eye(N, dtype=np.float32)
        st[IND0 + jj, jj * N:(jj + 1) * N] = 1.0
    st[ONESR, :] = 1.0
    return st.astype(BF16)


def _build_watt_pad(W_att):
    # w_att at column H of a [H, 2H] strip: window [H-c, 2H-c) has w_att at
    # relative column c only, so chunk c's score lands on PSUM partition c.
    wp = np.zeros((H, 2 * H), dtype=np.float32)
    wp[:, H] = W_att[:, 0]
    return wp.astype(BF16)


def _rep_mid(ap, n):
    """[p, h] -> [p, n, h] with the middle dim broadcast (step 0)."""
    p, h = ap.shape
    return ap.rearrange("k (u h) -> k u h", u=1).broadcast_to([p, n, h])


def _build_program():
    import concourse.mybir as mybir
    import concourse.tile as tile
    from concourse import bacc
    from contextlib import ExitStack

    f32 = mybir.dt.float32
    bf16 = mybir.dt.bfloat16
    i16 = mybir.dt.int16

    PB = _cache["PB"]         # padded edges per (core, batch)
    NGB = PB // 128           # gather column blocks per segment
    IWB = 2 * PB // 16        # idx cols per batch in the wrapped tile
    LW = NCHUNK * H           # big-lhsT width: 20 slices of 128
    BLOBW = H + 2 * NB * N + 2 * H   # wa | lft | lftq | watt

    nc = bacc.Bacc(
        "TRN2",
        target_bir_lowering=False,
        debug=False,
        enable_asserts=False,
        num_devices=NCORES,
    )

    # ---- DRAM I/O ----
    bf_d = nc.dram_tensor("bf_t", [NB, BIN, N * N], bf16, kind="ExternalInput").ap()
    blob_d = nc.dram_tensor("blob", [H, BLOBW], bf16, kind="ExternalInput").ap()
    batt_d = nc.dram_tensor("batt", [NCHUNK, 1], f32, kind="ExternalInput").ap()
    wbp_d = nc.dram_tensor("wbp", [1 + BIN, H], bf16, kind="ExternalInput").ap()
    lfj_d = nc.dram_tensor("lfj", [NCHUNK, CJ * NB * H], bf16,
                           kind="ExternalInput").ap()
    stat_d = nc.dram_tensor("statics", [NSTAT, CC], bf16, kind="ExternalInput").ap()
    tlf_d = nc.dram_tensor("tbl_lf", [NB * N, H], bf16, kind="ExternalInput").ap()
    ridx_d = nc.dram_tensor("ridx", [128, NB * IWB], i16, kind="ExternalInput").ap()
    out_d = nc.dram_tensor("out", [NB * PB, 2 * H], bf16, kind="ExternalOutput").ap()

    with tile.TileContext(nc) as tc, ExitStack() as ctx:
        const = ctx.enter_context(tc.tile_pool(name="const", bufs=1))
        stitched_p = ctx.enter_context(tc.tile_pool(name="stitched", bufs=1))
        big_p = ctx.enter_context(tc.tile_pool(name="biglhsT", bufs=1))
        perb_p = ctx.enter_context(tc.tile_pool(name="perb", bufs=2))
        r_p = ctx.enter_context(tc.tile_pool(name="relu", bufs=6))
        gath_p = ctx.enter_context(tc.tile_pool(name="gath", bufs=2))
        pre_psum = ctx.enter_context(tc.tile_pool(name="pre_ps", bufs=2, space="PSUM"))
        score_psum = ctx.enter_context(tc.tile_pool(name="sc_ps", bufs=2, space="PSUM"))
        misc_psum = ctx.enter_context(tc.tile_pool(name="mi_ps", bufs=1, space="PSUM"))
        dram_p = ctx.enter_context(tc.tile_pool(name="dram", bufs=1, space="DRAM"))

        # ---- one-time loads, ordered so batch 0's critical path starts
        # immediately (scheduler priority follows program order) ----
        blob_s = const.tile([H, BLOBW], bf16)
        nc.sync.dma_start(blob_s[:, 0:H + NB * N], blob_d[:, 0:H + NB * N])
        nc.sync.dma_start(blob_s[:, H + NB * N:], blob_d[:, H + NB * N:])
        wa_s = blob_s[:, 0:H]
        lft_s = blob_s[:, H:H + NB * N]
        lftq_s = blob_s[:, H + NB * N:H + 2 * NB * N]
        watt_s = blob_s[:, H + 2 * NB * N:]

        # stitched rhs tiles: static rows [0, NSTAT) + bf rows [WB0, K_TOT).
        # Statics are DMA'd into the first 500 columns, then replicated by
        # DVE/ACT; bf rows are disjoint so their DMAs never wait on the
        # replication.
        NST = 3
        st_tiles = []
        for t in range(NST):
            st_t = stitched_p.tile([K_TOT, HCOLS], bf16, tag=f"st{t}")
            st_tiles.append(st_t)

        def load_statics(t):
            nc.sync.dma_start(st_tiles[t][0:NSTAT, 0:CC], stat_d[:])

        def rep_statics(t):
            # replicate cols [CC, HCOLS) of the static rows, split ACT/DVE
            s = st_tiles[t]
            nc.scalar.copy(
                s[0:NSTAT, CC:4 * CC],
                _rep_mid(s[0:NSTAT, 0:CC], 3))
            nc.vector.tensor_copy(
                out=s[0:NSTAT, 4 * CC:HCOLS],
                in_=_rep_mid(s[0:NSTAT, 0:CC], HALFC - 4))

        def load_bf(b, h):
            nc.sync.dma_start(st_tiles[(2 * b + h) % NST][WB0:K_TOT, :],
                              bf_d[b, :, h * HCOLS:(h + 1) * HCOLS])

        load_statics(0)
        load_bf(0, 0)
        load_bf(0, 1)

        # persistent big-lhsT tiles, one per batch (Wb/bias rows + P rows
        # all filled at startup so no per-batch setup gates the chunk loops)
        big_tiles = []
        for t in range(NB):
            big_t = big_p.tile([K_TOT, LW], bf16, tag=f"big{t}")
            big_tiles.append(big_t)
        nc.sync.dma_start(big_tiles[0][ONESR:K_TOT, :], _rep_mid(wbp_d[:], NCHUNK))

        def p_setup(b):
            """P/Pq matmuls + big-tile row fills for batch b."""
            big = big_tiles[b]
            prep_ps = misc_psum.tile([N, 2 * H], f32, tag="prep")
            p_ps = prep_ps[:, 0:H]
            nc.tensor.matmul(p_ps, lft_s[:, b * N:(b + 1) * N], wa_s,
                             start=True, stop=True)
            nc.vector.tensor_copy(out=big[0:N, 0:H], in_=p_ps)
            # replicate P rows along the 20 slices; batch 0 is startup-
            # critical (DVE is idle then), the rest go to gpsimd's slack
            eng = nc.vector if b == 0 else nc.gpsimd
            eng.tensor_copy(
                out=big[0:N, H:LW],
                in_=_rep_mid(big[0:N, 0:H], NCHUNK - 1))
            # indicator rows: second P matmul with permuted output rows
            # (partition q = r*NCHUNK + c holds P[5c+r]); one DMA scatters
            # row-chunks of pq_s into big[IND0+r, c*H:(c+1)*H].
            pq_ps = prep_ps[:, H:2 * H]
            nc.tensor.matmul(pq_ps, lftq_s[:, b * N:(b + 1) * N], wa_s,
                             start=True, stop=True)
            pq_s = perb_p.tile([N, H], bf16, tag="pq")
            nc.vector.tensor_copy(out=pq_s[:], in_=pq_ps)
            nc.sync.dma_start(
                big[IND0:ONESR, :],
                pq_s[:].rearrange("(r c) h -> r (c h)", r=CJ))

        p_setup(0)
        rep_statics(0)
        for b in range(1, NB):
            p_setup(b)
            nc.sync.dma_start(big_tiles[b][ONESR:K_TOT, :],
                              _rep_mid(wbp_d[:], NCHUNK))

        # non-critical one-time loads go through gpsimd's SWDGE queue to
        # keep the HWDGE pipeline free for the critical-path DMAs
        tbl = dram_p.tile([NB * N, 2 * H], bf16)
        nc.gpsimd.dma_start(tbl[:, 0:H], tlf_d[:])
        ridx_s = const.tile([128, NB * IWB], i16)
        nc.gpsimd.dma_start(ridx_s[:], ridx_d[:])
        lfj_s = const.tile([NCHUNK, CJ * NB * H], bf16)
        nc.gpsimd.dma_start(lfj_s[:], lfj_d[:])
        batt_s = const.tile([NCHUNK, 1], f32)
        nc.gpsimd.dma_start(batt_s[:], batt_d[:])

        load_statics(1)
        rep_statics(1)

        for b in range(NB):
            big = big_tiles[b]

            if b == 0:
                load_bf(1, 0)
                load_bf(1, 1)
                load_statics(2)
                rep_statics(2)

            sc_ps = score_psum.tile([H, CC], f32, tag="sc")

            # score matmuls for group g are emitted after group g+1's pre
            # matmuls so PE fills the relu latency instead of stalling
            pend = None

            def emit_scores(rs, gix):
                for u in range(GROUP):
                    c = gix * GROUP + u
                    nc.tensor.matmul(sc_ps[:], watt_s[:, H - c:2 * H - c],
                                     rs[:, u * CC:(u + 1) * CC],
                                     start=(c == 0), stop=(c == NCHUNK - 1))

            for gidx in range(NCHUNK // GROUP):
                pre_ps = pre_psum.tile([H, GROUP * 512], f32, tag="pre")
                r_s = r_p.tile([H, GROUP * CC], bf16, tag="r")
                for u in range(GROUP):
                    c = gidx * GROUP + u
                    sthalf = st_tiles[(2 * b + c // HALFC) % NST]
                    col0 = (c % HALFC) * CC
                    nc.tensor.matmul(pre_ps[:, u * 512:u * 512 + CC],
                                     big[:, c * H:(c + 1) * H],
                                     sthalf[:, col0:col0 + CC],
                                     start=True, stop=True)
                if pend is not None:
                    emit_scores(*pend)
                # relu PSUM -> SBUF: both engines in parallel (latency)
                nc.scalar.activation(r_s[:, 0:CC], pre_ps[:, 0:CC],
                                     mybir.ActivationFunctionType.Relu)
                nc.vector.tensor_scalar_max(r_s[:, CC:2 * CC],
                                            pre_ps[:, 512:512 + CC], 0.0)
                pend = (r_s, gidx)
                if gidx == 6 and b < NB - 2:
                    load_bf(b + 2, 0)
                    load_bf(b + 2, 1)
            emit_scores(*pend)

            # sigmoid(score + b_att) on rows 0..19 -> [20(c), (jj, i)]
            sig_s = perb_p.tile([NCHUNK, CC], bf16, tag="sig")
            nc.scalar.activation(sig_s[:], sc_ps[0:NCHUNK, :],
                                 mybir.ActivationFunctionType.Sigmoid,
                                 bias=batt_s[:])
            # gf[b] = sum_jj att_jj^T @ lf rows {j = 5c+jj}: 5 accumulating
            # K=20 matmuls straight out of sig_s
            gf_ps = misc_psum.tile([N, H], f32, tag="gf")
            for jj in range(CJ):
                rhs = lfj_s[:, (jj * NB + b) * H:(jj * NB + b + 1) * H]
                nc.tensor.matmul(gf_ps[:], sig_s[:, jj * N:(jj + 1) * N], rhs,
                                 start=(jj == 0), stop=(jj == CJ - 1))
            gf_s = perb_p.tile([N, H], bf16, tag="gfs")
            nc.scalar.copy(gf_s[:], gf_ps[:])
            nc.scalar.dma_start(tbl[b * N:(b + 1) * N, H:2 * H], gf_s[:])

            # gather lf|gf rows for this batch's edges; first PB idxs are
            # r1 (-> out blocks [0,NGB)), next PB are r2.
            gout = gath_p.tile([128, 2 * NGB, 2 * H], bf16, tag="gout")
            nc.gpsimd.dma_gather(gout[:], tbl[b * N:(b + 1) * N, :],
                                 ridx_s[:, b * IWB:(b + 1) * IWB],
                                 2 * PB, 2 * PB, 2 * H, single_packet=False)
            gsum = gath_p.tile([128, NGB, 2 * H], bf16, tag="gsum")
            nc.vector.tensor_tensor(out=gsum[:], in0=gout[:, 0:NGB, :],
                                    in1=gout[:, NGB:2 * NGB, :],
                                    op=mybir.AluOpType.add)
            nc.scalar.dma_start(
                out_d[b * PB:(b + 1) * PB, :].rearrange(
                    "(p g) e -> p (g e)", p=128), gsum[:])

    nc.compile()
    return nc


def _host_prep(local_feats, binary_feats, sparse_idx, W_apair, W_binary,
               b_binary, W_att, b_att):
    """Shard + lay out inputs per core; returns (in_maps, unshard info)."""
    lf = np.asarray(local_feats, dtype=np.float32)
    bf = np.asarray(binary_feats, dtype=np.float32)
    si = np.asarray(sparse_idx)

    b_idx, i_idx, j_idx = si[:, 0], si[:, 1], si[:, 2]
    core = (b_idx // NB).astype(np.int64)
    lbat = (b_idx % NB).astype(np.int64)
    seg_rows = [[np.nonzero((core == k) & (lbat == b))[0]
                 for b in range(NB)] for k in range(NCORES)]
    maxc = max(len(r) for segs in seg_rows for r in segs)
    PB = max(128, ((maxc + 127) // 128) * 128)
    NGB = PB // 128
    _cache["PB"] = PB
    _cache["GPAD"] = NB * PB

    statics = _build_statics()
    watt = _build_watt_pad(np.asarray(W_att, dtype=np.float32))
    wbp = np.concatenate([np.asarray(b_binary, dtype=np.float32).reshape(1, H),
                          np.asarray(W_binary, dtype=np.float32)]).astype(BF16)
    batt = np.full((NCHUNK, 1), np.float32(np.asarray(b_att).reshape(-1)[0]),
                   dtype=np.float32)
    wa = np.asarray(W_apair, dtype=np.float32).astype(BF16)

    in_maps = []
    for k in range(NCORES):
        b0 = k * NB
        lf_k = lf[b0:b0 + NB].reshape(NB * N, H)
        lft_k = lf_k.T.astype(BF16)
        # column q of batch b = lf[b, CJ*(q%NCHUNK) + q//NCHUNK]
        qperm = (CJ * (np.arange(N) % NCHUNK) + np.arange(N) // NCHUNK)
        lftq_k = lf[b0:b0 + NB][:, qperm].reshape(NB * N, H).T.astype(BF16)
        # lfj tile: row c, col-block (g, b) holds lf[b, 5c+g]
        lfj_k = np.zeros((NCHUNK, CJ * NB * H), dtype=np.float32)
        for g in range(CJ):
            for c in range(NCHUNK):
                lfj_k[c, (g * NB) * H:(g * NB + NB) * H] = \
                    lf[b0:b0 + NB, CJ * c + g, :].reshape(-1)
        # [b, i, j, c] -> [b, c, (j, i)]
        bft_k = np.ascontiguousarray(
            bf[b0:b0 + NB].transpose(0, 3, 2, 1).reshape(NB, BIN, N * N)
        ).astype(BF16)
        # per-batch p-major gather index streams [r1 block | r2 block]
        ridx = np.zeros(NB * 2 * PB, dtype=np.int16)
        for b in range(NB):
            rows = seg_rows[k][b]
            r1 = np.zeros(PB, dtype=np.int16)
            r2 = np.zeros(PB, dtype=np.int16)
            r1[:len(rows)] = i_idx[rows].astype(np.int16)
            r2[:len(rows)] = j_idx[rows].astype(np.int16)
            # slot [p, g] <- idx[g*128+p]; out row p*NGB+g = seg position
            a1 = r1.reshape(128, NGB).T.reshape(-1)
            a2 = r2.reshape(128, NGB).T.reshape(-1)
            ridx[b * 2 * PB:b * 2 * PB + PB] = a1
            ridx[b * 2 * PB + PB:(b + 1) * 2 * PB] = a2
        blob = np.concatenate([wa, lft_k, lftq_k, watt], axis=1)
        in_maps.append({
            "bf_t": bft_k, "blob": blob, "batt": batt, "wbp": wbp,
            "lfj": lfj_k.astype(BF16), "statics": statics,
            "tbl_lf": lf_k.astype(BF16),
            "ridx": _wrap_idx(ridx),
        })
    return in_maps, seg_rows, PB


def _wrap_idx(idx):
    # int16 indices wrapped in 16 partitions (idx[k] at [k % 16, k // 16]),
    # replicated to all 8 gpsimd cores' partition groups.
    w = idx.reshape(-1, 16).T
    return np.ascontiguousarray(np.tile(w, (8, 1)))


def kernel(local_feats, binary_feats, sparse_idx, W_apair, W_binary,
           b_binary, W_att, b_att):
    in_maps, seg_rows, PB = _host_prep(
        local_feats, binary_feats, sparse_idx, W_apair, W_binary,
        b_binary, W_att, b_att)

    key = ("prog", NB * PB)
    if key not in _cache:
        _cache[key] = _build_program()
    nc = _cache[key]

    from concourse.bass_utils import run_bass_kernel_spmd
    trace = os.environ.get("KERNEL_TRACE", "0") == "1"
    res = run_bass_kernel_spmd(nc, in_maps, core_ids=list(range(NCORES)),
                               trace=trace)
    if trace and res.exec_time_ns is not None:
        print(f"HW exec time: {res.exec_time_ns} ns")
        print(f"mean exec time: {res.mean_exec_time_ns} ns")
        if res.instructions_and_trace is not None:
            print(f"trace: {res.instructions_and_trace[1]}")

    lp_full = np.empty((E, H), dtype=np.float32)
    gp_full = np.empty((E, H), dtype=np.float32)
    for k in range(NCORES):
        out = np.asarray(res.results[k]["out"]).astype(np.float32)
        for b in range(NB):
            rows = seg_rows[k][b]
            seg = out[b * PB:b * PB + len(rows)]
            lp_full[rows] = seg[:, 0:H]
            gp_full[rows] = seg[:, H:2 * H]
    return (lp_full, gp_full)


# revision 19
# speedup vs baseline: 1.3522x; 1.1156x over previous
"""Trainium2 Bass kernel for nn_Attention_6820408066818 (gnn message passing).

Math (reference):
  local_pair[b,i,j,:] = lf[b,i,:] + lf[b,j,:]
  att = relu(local_pair @ Wa + bf @ Wbin + b_bin)          # [B,N,N,H]
  score = sigmoid(att @ w_att + b_att)                     # [B,N,N,1]
  gf[b,i,:] = sum_j score[b,i,j] * lf[b,j,:]               # [B,N,H]
  out1[e] = local_pair[be,ie,je]   out2[e] = gf[be,ie] + gf[be,je]

Key identity: local_pair @ Wa = P[i] + P[j] with P = lf @ Wa, so the big
[B,N,N,H] tensor is never materialized.  Per core (4 batches), everything is
computed in [H=128 partitions, (j,i) columns] layout; "pre" is produced by a
single K=122 matmul per 500-column chunk whose stationary operand packs, per
chunk c (j in [5c,5c+5)):
    K rows  0- 99 : P[i] rows               <- identity(i) rhs rows
    K rows 100-104: P[5c+r] rows            <- j-indicator rhs rows
    K row  105    : b_binary                <- all-ones rhs row
    K rows 106-121: Wbin                    <- bf^T rhs rows (c contraction)
The whole matmul pipeline runs in bf16 (tolerance is 2e-2); accumulation
stays fp32 in PSUM.  score: K=128 matmul against a padded w_att so chunk c
lands on PSUM partition c; gf: one K=100 matmul from a (jj,c)-ordered
sigmoid tile; sparse gather per batch from a combined [lf|gf] bf16 row
table so each batch's edge chains overlap the next batch's compute.

Sharding: data-parallel over batch, 4 batches per core, 8 cores.
"""

import os
import sys

import numpy as np
import ml_dtypes

sys.path.insert(0, "/opt/trn_rl_repo")

B, N, H, BIN, E = 32, 100, 128, 16, 20000
NCORES = 8
NB = B // NCORES          # batches per core
CJ = 5                    # j's per chunk
CC = CJ * N               # 500 columns per chunk
NCHUNK = N // CJ          # 20 chunks per batch
HALFC = NCHUNK // 2       # chunks per stitched tile
HCOLS = HALFC * CC        # 5000
GROUP = 2                 # chunks per relu group

# K-row layout
IND0 = N                  # j-indicator rows at [100, 105)
ONESR = N + CJ            # 105: all-ones rhs row <-> b_binary lhsT row
WB0 = ONESR + 1           # Wbin rows at [106, 122)
K_TOT = WB0 + BIN         # 122
NSTAT = ONESR + 1         # 106 static rhs rows (identity+ind+ones)

BF16 = ml_dtypes.bfloat16

_cache = {}


def _build_statics():
    """Static rhs rows [NSTAT, CC]; bf rows live at [WB0, K_TOT) below."""
    st = np.zeros((NSTAT, CC), dtype=np.float32)
    for jj in range(CJ):
        st[:N, jj * N:(jj + 1) * N] = np.eye(N, dtype=np.float32)
        st[IND0 + jj, jj * N:(jj + 1) * N] = 1.0
    st[ONESR, :] = 1.0
    return st.astype(BF16)


def _build_watt_pad(W_att):
    # w_att at column H of a [H, 2H] strip: window [H-c, 2H-c) has w_att at
    # relative column c only, so chunk c's score lands on PSUM partition c.
    wp = np.zeros((H, 2 * H), dtype=np.float32)
    wp[:, H] = W_att[:, 0]
    return wp.astype(BF16)


def _rep_mid(ap, n):
    """[p, h] -> [p, n, h] with the middle dim broadcast (step 0)."""
    p, h = ap.shape
    return ap.rearrange("k (u h) -> k u h", u=1).broadcast_to([p, n, h])


def _build_program():
    import concourse.mybir as mybir
    import concourse.tile as tile
    from concourse import bacc
    from contextlib import ExitStack

    f32 = mybir.dt.float32
    bf16 = mybir.dt.bfloat16
    i16 = mybir.dt.int16

    PB = _cache["PB"]         # padded edges per (core, batch)
    NGB = PB // 128           # gather column blocks per segment
    IWB = 2 * PB // 16        # idx cols per batch in the wrapped tile
    LW = NCHUNK * H           # big-lhsT width: 20 slices of 128
    BLOBW = H + 2 * NB * N + 2 * H   # wa | lft | lftq | watt

    nc = bacc.Bacc(
        "TRN2",
        target_bir_lowering=False,
        debug=False,
        enable_asserts=False,
        num_devices=NCORES,
    )

    # ---- DRAM I/O ----
    bf_d = nc.dram_tensor("bf_t", [NB, BIN, N * N], bf16, kind="ExternalInput").ap()
    blob_d = nc.dram_tensor("blob", [H, BLOBW], bf16, kind="ExternalInput").ap()
    batt_d = nc.dram_tensor("batt", [NCHUNK, 1], f32, kind="ExternalInput").ap()
    wbp_d = nc.dram_tensor("wbp", [1 + BIN, H], bf16, kind="ExternalInput").ap()
    lfj_d = nc.dram_tensor("lfj", [NCHUNK, CJ * NB * H], bf16,
                           kind="ExternalInput").ap()
    stat_d = nc.dram_tensor("statics", [NSTAT, CC], bf16, kind="ExternalInput").ap()
    tlf_d = nc.dram_tensor("tbl_lf", [NB * N, H], bf16, kind="ExternalInput").ap()
    ridx_d = nc.dram_tensor("ridx", [128, NB * IWB], i16, kind="ExternalInput").ap()
    out_d = nc.dram_tensor("out", [NB * PB, 2 * H], bf16, kind="ExternalOutput").ap()

    with tile.TileContext(nc) as tc, ExitStack() as ctx:
        const = ctx.enter_context(tc.tile_pool(name="const", bufs=1))
        stitched_p = ctx.enter_context(tc.tile_pool(name="stitched", bufs=1))
        big_p = ctx.enter_context(tc.tile_pool(name="biglhsT", bufs=1))
        perb_p = ctx.enter_context(tc.tile_pool(name="perb", bufs=2))
        r_p = ctx.enter_context(tc.tile_pool(name="relu", bufs=6))
        gath_p = ctx.enter_context(tc.tile_pool(name="gath", bufs=2))
        pre_psum = ctx.enter_context(tc.tile_pool(name="pre_ps", bufs=2, space="PSUM"))
        score_psum = ctx.enter_context(tc.tile_pool(name="sc_ps", bufs=2, space="PSUM"))
        misc_psum = ctx.enter_context(tc.tile_pool(name="mi_ps", bufs=1, space="PSUM"))
        dram_p = ctx.enter_context(tc.tile_pool(name="dram", bufs=1, space="DRAM"))

        # ---- one-time loads, ordered so batch 0's critical path starts
        # immediately (scheduler priority follows program order) ----
        blob_s = const.tile([H, BLOBW], bf16)
        nc.sync.dma_start(blob_s[:, 0:H + NB * N], blob_d[:, 0:H + NB * N])
        nc.sync.dma_start(blob_s[:, H + NB * N:], blob_d[:, H + NB * N:])
        wa_s = blob_s[:, 0:H]
        lft_s = blob_s[:, H:H + NB * N]
        lftq_s = blob_s[:, H + NB * N:H + 2 * NB * N]
        watt_s = blob_s[:, H + 2 * NB * N:]

        # stitched rhs tiles: static rows [0, NSTAT) + bf rows [WB0, K_TOT).
        # Statics are DMA'd into the first 500 columns, then replicated by
        # DVE/ACT; bf rows are disjoint so their DMAs never wait on the
        # replication.
        NST = 3
        st_tiles = []
        for t in range(NST):
            st_t = stitched_p.tile([K_TOT, HCOLS], bf16, tag=f"st{t}")
            st_tiles.append(st_t)

        def load_statics(t):
            nc.sync.dma_start(st_tiles[t][0:NSTAT, 0:CC], stat_d[:])

        def rep_statics_act(t):
            s = st_tiles[t]
            nc.scalar.copy(
                s[0:NSTAT, CC:4 * CC],
                _rep_mid(s[0:NSTAT, 0:CC], 3))

        def rep_statics_dve(t):
            s = st_tiles[t]
            nc.vector.tensor_copy(
                out=s[0:NSTAT, 4 * CC:HCOLS],
                in_=_rep_mid(s[0:NSTAT, 0:CC], HALFC - 4))

        def load_bf(b, h):
            nc.sync.dma_start(st_tiles[(2 * b + h) % NST][WB0:K_TOT, :],
                              bf_d[b, :, h * HCOLS:(h + 1) * HCOLS])

        load_statics(0)
        load_bf(0, 0)
        load_bf(0, 1)

        # persistent big-lhsT tiles, one per batch (Wb/bias rows + P rows
        # all filled before each batch starts; batch 1-3 setup is spread
        # through batch 0's chunk loop so its DMAs never crowd batch 0's)
        big_tiles = []
        for t in range(NB):
            big_t = big_p.tile([K_TOT, LW], bf16, tag=f"big{t}")
            big_tiles.append(big_t)
        nc.sync.dma_start(big_tiles[0][ONESR:K_TOT, :], _rep_mid(wbp_d[:], NCHUNK))

        def p_setup(b):
            """P/Pq matmuls + big-tile row fills for batch b."""
            big = big_tiles[b]
            prep_ps = misc_psum.tile([N, 2 * H], f32, tag="prep")
            p_ps = prep_ps[:, 0:H]
            nc.tensor.matmul(p_ps, lft_s[:, b * N:(b + 1) * N], wa_s,
                             start=True, stop=True)
            nc.vector.tensor_copy(out=big[0:N, 0:H], in_=p_ps)
            # replicate P rows along the 20 slices; batch 0 is startup-
            # critical (DVE is idle then), the rest go to gpsimd's slack
            eng = nc.vector if b == 0 else nc.gpsimd
            eng.tensor_copy(
                out=big[0:N, H:LW],
                in_=_rep_mid(big[0:N, 0:H], NCHUNK - 1))
            # indicator rows: second P matmul with permuted output rows
            # (partition q = r*NCHUNK + c holds P[5c+r]); one DMA scatters
            # row-chunks of pq_s into big[IND0+r, c*H:(c+1)*H].
            pq_ps = prep_ps[:, H:2 * H]
            nc.tensor.matmul(pq_ps, lftq_s[:, b * N:(b + 1) * N], wa_s,
                             start=True, stop=True)
            pq_s = perb_p.tile([N, H], bf16, tag="pq")
            nc.vector.tensor_copy(out=pq_s[:], in_=pq_ps)
            nc.sync.dma_start(
                big[IND0:ONESR, :],
                pq_s[:].rearrange("(r c) h -> r (c h)", r=CJ))
            if b > 0:
                nc.sync.dma_start(big_tiles[b][ONESR:K_TOT, :],
                                  _rep_mid(wbp_d[:], NCHUNK))

        p_setup(0)
        rep_statics_act(0)
        rep_statics_dve(0)

        load_statics(1)
        load_statics(2)
        load_bf(1, 0)

        # non-critical one-time loads go through gpsimd's SWDGE queue to
        # keep the HWDGE pipeline free for the critical-path DMAs
        tbl = dram_p.tile([NB * N, 2 * H], bf16)
        nc.gpsimd.dma_start(tbl[:, 0:H], tlf_d[:])
        ridx_s = const.tile([128, NB * IWB], i16)
        nc.gpsimd.dma_start(ridx_s[:], ridx_d[:])
        lfj_s = const.tile([NCHUNK, CJ * NB * H], bf16)
        nc.gpsimd.dma_start(lfj_s[:], lfj_d[:])
        batt_s = const.tile([NCHUNK, 1], f32)
        nc.gpsimd.dma_start(batt_s[:], batt_d[:])

        for b in range(NB):
            big = big_tiles[b]

            sc_ps = score_psum.tile([H, CC], f32, tag="sc")

            # score matmuls for group g are emitted after group g+1's pre
            # matmuls so PE fills the relu latency instead of stalling
            pend = None

            def emit_scores(rs, gix):
                for u in range(GROUP):
                    c = gix * GROUP + u
                    nc.tensor.matmul(sc_ps[:], watt_s[:, H - c:2 * H - c],
                                     rs[:, u * CC:(u + 1) * CC],
                                     start=(c == 0), stop=(c == NCHUNK - 1))

            for gidx in range(NCHUNK // GROUP):
                pre_ps = pre_psum.tile([H, GROUP * 512], f32, tag="pre")
                r_s = r_p.tile([H, GROUP * CC], bf16, tag="r")
                for u in range(GROUP):
                    c = gidx * GROUP + u
                    sthalf = st_tiles[(2 * b + c // HALFC) % NST]
                    col0 = (c % HALFC) * CC
                    nc.tensor.matmul(pre_ps[:, u * 512:u * 512 + CC],
                                     big[:, c * H:(c + 1) * H],
                                     sthalf[:, col0:col0 + CC],
                                     start=True, stop=True)
                if pend is not None:
                    emit_scores(*pend)
                # relu PSUM -> SBUF: both engines in parallel (latency)
                nc.scalar.activation(r_s[:, 0:CC], pre_ps[:, 0:CC],
                                     mybir.ActivationFunctionType.Relu)
                nc.vector.tensor_scalar_max(r_s[:, CC:2 * CC],
                                            pre_ps[:, 512:512 + CC], 0.0)
                pend = (r_s, gidx)
                if b == 0:
                    # spread the remaining setup through batch 0 so each
                    # piece's deps land just before the engines reach it
                    if gidx in (1, 3, 5):
                        p_setup(gidx // 2 + 1)
                    elif gidx == 2:
                        rep_statics_act(1)
                        rep_statics_dve(1)
                    elif gidx == 6:
                        rep_statics_act(2)
                        rep_statics_dve(2)
                        load_bf(1, 1)
                        load_bf(2, 0)
                        load_bf(2, 1)
                elif b == 1 and gidx == 6:
                    load_bf(3, 0)
                    load_bf(3, 1)
            emit_scores(*pend)

            # sigmoid(score + b_att) on rows 0..19 -> [20(c), (jj, i)]
            sig_s = perb_p.tile([NCHUNK, CC], bf16, tag="sig")
            nc.scalar.activation(sig_s[:], sc_ps[0:NCHUNK, :],
                                 mybir.ActivationFunctionType.Sigmoid,
                                 bias=batt_s[:])
            # gf[b] = sum_jj att_jj^T @ lf rows {j = 5c+jj}: 5 accumulating
            # K=20 matmuls straight out of sig_s (reuses the prep PSUM bank)
            gf_full = misc_psum.tile([N, 2 * H], f32, tag="prep")
            gf_ps = gf_full[:, 0:H]
            for jj in range(CJ):
                rhs = lfj_s[:, (jj * NB + b) * H:(jj * NB + b + 1) * H]
                nc.tensor.matmul(gf_ps[:], sig_s[:, jj * N:(jj + 1) * N], rhs,
                                 start=(jj == 0), stop=(jj == CJ - 1))
            gf_s = perb_p.tile([N, H], bf16, tag="gfs")
            nc.scalar.copy(gf_s[:], gf_ps[:])
            nc.scalar.dma_start(tbl[b * N:(b + 1) * N, H:2 * H], gf_s[:])

            # gather lf|gf rows for this batch's edges; first PB idxs are
            # r1 (-> out blocks [0,NGB)), next PB are r2.
            gout = gath_p.tile([128, 2 * NGB, 2 * H], bf16, tag="gout")
            nc.gpsimd.dma_gather(gout[:], tbl[b * N:(b + 1) * N, :],
                                 ridx_s[:, b * IWB:(b + 1) * IWB],
                                 2 * PB, 2 * PB, 2 * H, single_packet=False)
            gsum = gath_p.tile([128, NGB, 2 * H], bf16, tag="gsum")
            nc.vector.tensor_tensor(out=gsum[:], in0=gout[:, 0:NGB, :],
                                    in1=gout[:, NGB:2 * NGB, :],
                                    op=mybir.AluOpType.add)
            nc.sync.dma_start(
                out_d[b * PB:(b + 1) * PB, :].rearrange(
                    "(p g) e -> p (g e)", p=128), gsum[:])

    nc.compile()
    return nc


def _host_prep(local_feats, binary_feats, sparse_idx, W_apair, W_binary,
               b_binary, W_att, b_att):
    """Shard + lay out inputs per core; returns (in_maps, unshard info)."""
    lf = np.asarray(local_feats, dtype=np.float32)
    bf = np.asarray(binary_feats, dtype=np.float32)
    si = np.asarray(sparse_idx)

    b_idx, i_idx, j_idx = si[:, 0], si[:, 1], si[:, 2]
    core = (b_idx // NB).astype(np.int64)
    lbat = (b_idx % NB).astype(np.int64)
    seg_rows = [[np.nonzero((core == k) & (lbat == b))[0]
                 for b in range(NB)] for k in range(NCORES)]
    maxc = max(len(r) for segs in seg_rows for r in segs)
    PB = max(128, ((maxc + 127) // 128) * 128)
    NGB = PB // 128
    _cache["PB"] = PB
    _cache["GPAD"] = NB * PB

    statics = _build_statics()
    watt = _build_watt_pad(np.asarray(W_att, dtype=np.float32))
    wbp = np.concatenate([np.asarray(b_binary, dtype=np.float32).reshape(1, H),
                          np.asarray(W_binary, dtype=np.float32)]).astype(BF16)
    batt = np.full((NCHUNK, 1), np.float32(np.asarray(b_att).reshape(-1)[0]),
                   dtype=np.float32)
    wa = np.asarray(W_apair, dtype=np.float32).astype(BF16)

    in_maps = []
    for k in range(NCORES):
        b0 = k * NB
        lf_k = lf[b0:b0 + NB].reshape(NB * N, H)
        lft_k = lf_k.T.astype(BF16)
        # column q of batch b = lf[b, CJ*(q%NCHUNK) + q//NCHUNK]
        qperm = (CJ * (np.arange(N) % NCHUNK) + np.arange(N) // NCHUNK)
        lftq_k = lf[b0:b0 + NB][:, qperm].reshape(NB * N, H).T.astype(BF16)
        # lfj tile: row c, col-block (g, b) holds lf[b, 5c+g]
        lfj_k = np.zeros((NCHUNK, CJ * NB * H), dtype=np.float32)
        for g in range(CJ):
            for c in range(NCHUNK):
                lfj_k[c, (g * NB) * H:(g * NB + NB) * H] = \
                    lf[b0:b0 + NB, CJ * c + g, :].reshape(-1)
        # [b, i, j, c] -> [b, c, (j, i)]
        bft_k = np.ascontiguousarray(
            bf[b0:b0 + NB].transpose(0, 3, 2, 1).reshape(NB, BIN, N * N)
        ).astype(BF16)
        # per-batch p-major gather index streams [r1 block | r2 block]
        ridx = np.zeros(NB * 2 * PB, dtype=np.int16)
        for b in range(NB):
            rows = seg_rows[k][b]
            r1 = np.zeros(PB, dtype=np.int16)
            r2 = np.zeros(PB, dtype=np.int16)
            r1[:len(rows)] = i_idx[rows].astype(np.int16)
            r2[:len(rows)] = j_idx[rows].astype(np.int16)
            # slot [p, g] <- idx[g*128+p]; out row p*NGB+g = seg position
            a1 = r1.reshape(128, NGB).T.reshape(-1)
            a2 = r2.reshape(128, NGB).T.reshape(-1)
            ridx[b * 2 * PB:b * 2 * PB + PB] = a1
            ridx[b * 2 * PB + PB:(b + 1) * 2 * PB] = a2
        blob = np.concatenate([wa, lft_k, lftq_k, watt], axis=1)
        in_maps.append({
            "bf_t": bft_k, "blob": blob, "batt": batt, "wbp": wbp,
            "lfj": lfj_k.astype(BF16), "statics": statics,
            "tbl_lf": lf_k.astype(BF16),
            "ridx": _wrap_idx(ridx),
        })
    return in_maps, seg_rows, PB


def _wrap_idx(idx):
    # int16 indices wrapped in 16 partitions (idx[k] at [k % 16, k // 16]),
    # replicated to all 8 gpsimd cores' partition groups.
    w = idx.reshape(-1, 16).T
    return np.ascontiguousarray(np.tile(w, (8, 1)))


def kernel(local_feats, binary_feats, sparse_idx, W_apair, W_binary,
           b_binary, W_att, b_att):
    in_maps, seg_rows, PB = _host_prep(
        local_feats, binary_feats, sparse_idx, W_apair, W_binary,
        b_binary, W_att, b_att)

    key = ("prog", NB * PB)
    if key not in _cache:
        _cache[key] = _build_program()
    nc = _cache[key]

    from concourse.bass_utils import run_bass_kernel_spmd
    trace = os.environ.get("KERNEL_TRACE", "0") == "1"
    res = run_bass_kernel_spmd(nc, in_maps, core_ids=list(range(NCORES)),
                               trace=trace)
    if trace and res.exec_time_ns is not None:
        print(f"HW exec time: {res.exec_time_ns} ns")
        print(f"mean exec time: {res.mean_exec_time_ns} ns")
        if res.instructions_and_trace is not None:
            print(f"trace: {res.instructions_and_trace[1]}")

    lp_full = np.empty((E, H), dtype=np.float32)
    gp_full = np.empty((E, H), dtype=np.float32)
    for k in range(NCORES):
        out = np.asarray(res.results[k]["out"]).astype(np.float32)
        for b in range(NB):
            rows = seg_rows[k][b]
            seg = out[b * PB:b * PB + len(rows)]
            lp_full[rows] = seg[:, 0:H]
            gp_full[rows] = seg[:, H:2 * H]
    return (lp_full, gp_full)


# revision 23
# speedup vs baseline: 1.3599x; 1.0057x over previous
"""Trainium2 Bass kernel for nn_Attention_6820408066818 (gnn message passing).

Math (reference):
  local_pair[b,i,j,:] = lf[b,i,:] + lf[b,j,:]
  att = relu(local_pair @ Wa + bf @ Wbin + b_bin)          # [B,N,N,H]
  score = sigmoid(att @ w_att + b_att)                     # [B,N,N,1]
  gf[b,i,:] = sum_j score[b,i,j] * lf[b,j,:]               # [B,N,H]
  out1[e] = local_pair[be,ie,je]   out2[e] = gf[be,ie] + gf[be,je]

Key identity: local_pair @ Wa = P[i] + P[j] with P = lf @ Wa, so the big
[B,N,N,H] tensor is never materialized.  Per core (4 batches), everything is
computed in [H=128 partitions, (j,i) columns] layout; "pre" is produced by a
single K=122 matmul per 500-column chunk whose stationary operand packs, per
chunk c (j in [5c,5c+5)):
    K rows  0- 99 : P[i] rows               <- identity(i) rhs rows
    K rows 100-104: P[5c+r] rows            <- j-indicator rhs rows
    K row  105    : b_binary                <- all-ones rhs row
    K rows 106-121: Wbin                    <- bf^T rhs rows (c contraction)
The whole matmul pipeline runs in bf16 (tolerance is 2e-2); accumulation
stays fp32 in PSUM.  score: K=128 matmul against a padded w_att so chunk c
lands on PSUM partition c; gf: one K=100 matmul from a (jj,c)-ordered
sigmoid tile; sparse gather per batch from a combined [lf|gf] bf16 row
table so each batch's edge chains overlap the next batch's compute.

Sharding: data-parallel over batch, 4 batches per core, 8 cores.
"""

import os
import sys

import numpy as np
import ml_dtypes

sys.path.insert(0, "/opt/trn_rl_repo")

B, N, H, BIN, E = 32, 100, 128, 16, 20000
NCORES = 8
NB = B // NCORES          # batches per core
CJ = 5                    # j's per chunk
CC = CJ * N               # 500 columns per chunk
NCHUNK = N // CJ          # 20 chunks per batch
HALFC = NCHUNK // 2       # chunks per stitched tile
HCOLS = HALFC * CC        # 5000
GROUP = 2                 # chunks per relu group

# K-row layout
IND0 = N                  # j-indicator rows at [100, 105)
ONESR = N + CJ            # 105: all-ones rhs row <-> b_binary lhsT row
WB0 = ONESR + 1           # Wbin rows at [106, 122)
K_TOT = WB0 + BIN         # 122
NSTAT = ONESR + 1         # 106 static rhs rows (identity+ind+ones)

BF16 = ml_dtypes.bfloat16

_cache = {}


def _build_statics():
    """Static rhs rows [NSTAT, CC]; bf rows live at [WB0, K_TOT) below."""
    st = np.zeros((NSTAT, CC), dtype=np.float32)
    for jj in range(CJ):
        st[:N, jj * N:(jj + 1) * N] = np.eye(N, dtype=np.float32)
        st[IND0 + jj, jj * N:(jj + 1) * N] = 1.0
    st[ONESR, :] = 1.0
    return st.astype(BF16)


def _build_watt_pad(W_att):
    # w_att at column H of a [H, 2H] strip: window [H-c, 2H-c) has w_att at
    # relative column c only, so chunk c's score lands on PSUM partition c.
    wp = np.zeros((H, 2 * H), dtype=np.float32)
    wp[:, H] = W_att[:, 0]
    return wp.astype(BF16)


def _rep_mid(ap, n):
    """[p, h] -> [p, n, h] with the middle dim broadcast (step 0)."""
    p, h = ap.shape
    return ap.rearrange("k (u h) -> k u h", u=1).broadcast_to([p, n, h])


def _build_program():
    import concourse.mybir as mybir
    import concourse.tile as tile
    from concourse import bacc
    from contextlib import ExitStack

    f32 = mybir.dt.float32
    bf16 = mybir.dt.bfloat16
    i16 = mybir.dt.int16

    PB = _cache["PB"]         # padded edges per (core, batch)
    NGB = PB // 128           # gather column blocks per segment
    IWB = 2 * PB // 16        # idx cols per batch in the wrapped tile
    LW = NCHUNK * H           # big-lhsT width: 20 slices of 128
    BLOBW = H + 2 * NB * N + 2 * H   # wa | lft | lftq | watt

    nc = bacc.Bacc(
        "TRN2",
        target_bir_lowering=False,
        debug=False,
        enable_asserts=False,
        num_devices=NCORES,
    )

    # ---- DRAM I/O ----
    bf_d = nc.dram_tensor("bf_t", [NB, BIN, N * N], bf16, kind="ExternalInput").ap()
    blob_d = nc.dram_tensor("blob", [H, BLOBW], bf16, kind="ExternalInput").ap()
    batt_d = nc.dram_tensor("batt", [NCHUNK, 1], f32, kind="ExternalInput").ap()
    wbp_d = nc.dram_tensor("wbp", [1 + BIN, H], bf16, kind="ExternalInput").ap()
    lfj_d = nc.dram_tensor("lfj", [NCHUNK, CJ * NB * H], bf16,
                           kind="ExternalInput").ap()
    stat_d = nc.dram_tensor("statics", [NSTAT, CC], bf16, kind="ExternalInput").ap()
    tlf_d = nc.dram_tensor("tbl_lf", [NB * N, H], bf16, kind="ExternalInput").ap()
    ridx_d = nc.dram_tensor("ridx", [128, NB * IWB], i16, kind="ExternalInput").ap()
    out_d = nc.dram_tensor("out", [NB * PB, 2 * H], bf16, kind="ExternalOutput").ap()

    with tile.TileContext(nc) as tc, ExitStack() as ctx:
        const = ctx.enter_context(tc.tile_pool(name="const", bufs=1))
        stitched_p = ctx.enter_context(tc.tile_pool(name="stitched", bufs=1))
        big_p = ctx.enter_context(tc.tile_pool(name="biglhsT", bufs=1))
        perb_p = ctx.enter_context(tc.tile_pool(name="perb", bufs=2))
        r_p = ctx.enter_context(tc.tile_pool(name="relu", bufs=6))
        gath_p = ctx.enter_context(tc.tile_pool(name="gath", bufs=2))
        pre_psum = ctx.enter_context(tc.tile_pool(name="pre_ps", bufs=2, space="PSUM"))
        score_psum = ctx.enter_context(tc.tile_pool(name="sc_ps", bufs=2, space="PSUM"))
        misc_psum = ctx.enter_context(tc.tile_pool(name="mi_ps", bufs=1, space="PSUM"))
        dram_p = ctx.enter_context(tc.tile_pool(name="dram", bufs=1, space="DRAM"))

        # ---- one-time loads, ordered so batch 0's critical path starts
        # immediately (scheduler priority follows program order) ----
        blob_s = const.tile([H, BLOBW], bf16)
        nc.sync.dma_start(blob_s[:, 0:H + NB * N], blob_d[:, 0:H + NB * N])
        nc.sync.dma_start(blob_s[:, H + NB * N:], blob_d[:, H + NB * N:])
        wa_s = blob_s[:, 0:H]
        lft_s = blob_s[:, H:H + NB * N]
        lftq_s = blob_s[:, H + NB * N:H + 2 * NB * N]
        watt_s = blob_s[:, H + 2 * NB * N:]

        # stitched rhs tiles: static rows [0, NSTAT) + bf rows [WB0, K_TOT).
        # Statics are DMA'd into the first 500 columns, then replicated by
        # DVE/ACT; bf rows are disjoint so their DMAs never wait on the
        # replication.
        NST = 3
        st_tiles = []
        for t in range(NST):
            st_t = stitched_p.tile([K_TOT, HCOLS], bf16, tag=f"st{t}")
            st_tiles.append(st_t)

        def load_statics(t):
            nc.sync.dma_start(st_tiles[t][0:NSTAT, 0:CC], stat_d[:])

        def rep_statics_act(t):
            s = st_tiles[t]
            nc.scalar.copy(
                s[0:NSTAT, CC:4 * CC],
                _rep_mid(s[0:NSTAT, 0:CC], 3))

        def rep_statics_dve(t):
            s = st_tiles[t]
            nc.vector.tensor_copy(
                out=s[0:NSTAT, 4 * CC:HCOLS],
                in_=_rep_mid(s[0:NSTAT, 0:CC], HALFC - 4))

        def load_bf(b, h):
            nc.sync.dma_start(st_tiles[(2 * b + h) % NST][WB0:K_TOT, :],
                              bf_d[b, :, h * HCOLS:(h + 1) * HCOLS])

        load_statics(0)
        load_bf(0, 0)

        # persistent big-lhsT tiles, one per batch (Wb/bias rows + P rows
        # all filled before each batch starts; batch 1-3 setup is spread
        # through batch 0's chunk loop so its DMAs never crowd batch 0's)
        big_tiles = []
        for t in range(NB):
            big_t = big_p.tile([K_TOT, LW], bf16, tag=f"big{t}")
            big_tiles.append(big_t)
        nc.sync.dma_start(big_tiles[0][ONESR:K_TOT, :], _rep_mid(wbp_d[:], NCHUNK))

        def p_setup(b):
            """P/Pq matmuls + big-tile row fills for batch b."""
            big = big_tiles[b]
            prep_ps = misc_psum.tile([N, 2 * H], f32, tag="prep")
            p_ps = prep_ps[:, 0:H]
            nc.tensor.matmul(p_ps, lft_s[:, b * N:(b + 1) * N], wa_s,
                             start=True, stop=True)
            # indicator rows: second P matmul with permuted output rows
            # (partition q = r*NCHUNK + c holds P[5c+r]); one DMA scatters
            # row-chunks of pq_s into big[IND0+r, c*H:(c+1)*H].  The pq copy
            # runs before the P-row replication so that DMA issues ASAP.
            pq_ps = prep_ps[:, H:2 * H]
            nc.tensor.matmul(pq_ps, lftq_s[:, b * N:(b + 1) * N], wa_s,
                             start=True, stop=True)
            nc.vector.tensor_copy(out=big[0:N, 0:H], in_=p_ps)
            pq_s = perb_p.tile([N, H], bf16, tag="pq")
            nc.vector.tensor_copy(out=pq_s[:], in_=pq_ps)
            nc.sync.dma_start(
                big[IND0:ONESR, :],
                pq_s[:].rearrange("(r c) h -> r (c h)", r=CJ))
            # replicate P rows along the 20 slices; batch 0 is startup-
            # critical (DVE is idle then), the rest go to gpsimd's slack
            eng = nc.vector if b == 0 else nc.gpsimd
            eng.tensor_copy(
                out=big[0:N, H:LW],
                in_=_rep_mid(big[0:N, 0:H], NCHUNK - 1))
            if b > 0:
                nc.sync.dma_start(big_tiles[b][ONESR:K_TOT, :],
                                  _rep_mid(wbp_d[:], NCHUNK))

        p_setup(0)
        rep_statics_act(0)
        rep_statics_dve(0)

        load_bf(0, 1)
        load_statics(1)
        load_statics(2)
        load_bf(1, 0)

        # non-critical one-time loads go through gpsimd's SWDGE queue to
        # keep the HWDGE pipeline free for the critical-path DMAs
        tbl = dram_p.tile([NB * N, 2 * H], bf16)
        nc.gpsimd.dma_start(tbl[:, 0:H], tlf_d[:])
        ridx_s = const.tile([128, NB * IWB], i16)
        nc.gpsimd.dma_start(ridx_s[:], ridx_d[:])
        lfj_s = const.tile([NCHUNK, CJ * NB * H], bf16)
        nc.gpsimd.dma_start(lfj_s[:], lfj_d[:])
        batt_s = const.tile([NCHUNK, 1], f32)
        nc.gpsimd.dma_start(batt_s[:], batt_d[:])

        for b in range(NB):
            big = big_tiles[b]

            sc_ps = score_psum.tile([H, CC], f32, tag="sc")

            # score matmuls for group g are emitted after group g+1's pre
            # matmuls so PE fills the relu latency instead of stalling
            pend = None

            def emit_scores(rs, gix):
                for u in range(GROUP):
                    c = gix * GROUP + u
                    nc.tensor.matmul(sc_ps[:], watt_s[:, H - c:2 * H - c],
                                     rs[:, u * CC:(u + 1) * CC],
                                     start=(c == 0), stop=(c == NCHUNK - 1))

            for gidx in range(NCHUNK // GROUP):
                pre_ps = pre_psum.tile([H, GROUP * 512], f32, tag="pre")
                r_s = r_p.tile([H, GROUP * CC], bf16, tag="r")
                for u in range(GROUP):
                    c = gidx * GROUP + u
                    sthalf = st_tiles[(2 * b + c // HALFC) % NST]
                    col0 = (c % HALFC) * CC
                    nc.tensor.matmul(pre_ps[:, u * 512:u * 512 + CC],
                                     big[:, c * H:(c + 1) * H],
                                     sthalf[:, col0:col0 + CC],
                                     start=True, stop=True)
                if pend is not None:
                    emit_scores(*pend)
                # relu PSUM -> SBUF: both engines in parallel (latency)
                nc.scalar.activation(r_s[:, 0:CC], pre_ps[:, 0:CC],
                                     mybir.ActivationFunctionType.Relu)
                nc.vector.tensor_scalar_max(r_s[:, CC:2 * CC],
                                            pre_ps[:, 512:512 + CC], 0.0)
                pend = (r_s, gidx)
                if b == 0:
                    # spread the remaining setup through batch 0 so each
                    # piece's deps land just before the engines reach it
                    if gidx in (1, 3, 5):
                        p_setup(gidx // 2 + 1)
                    elif gidx == 2:
                        rep_statics_act(1)
                        rep_statics_dve(1)
                    elif gidx == 6:
                        rep_statics_act(2)
                        rep_statics_dve(2)
                        load_bf(1, 1)
                        load_bf(2, 0)
                        load_bf(2, 1)
                elif b == 1 and gidx == 6:
                    load_bf(3, 0)
                    load_bf(3, 1)
            emit_scores(*pend)

            # sigmoid(score + b_att) on rows 0..19 -> [20(c), (jj, i)]
            sig_s = perb_p.tile([NCHUNK, CC], bf16, tag="sig")
            nc.scalar.activation(sig_s[:], sc_ps[0:NCHUNK, :],
                                 mybir.ActivationFunctionType.Sigmoid,
                                 bias=batt_s[:])
            # gf[b] = sum_jj att_jj^T @ lf rows {j = 5c+jj}: 5 accumulating
            # K=20 matmuls straight out of sig_s (reuses the prep PSUM bank)
            gf_full = misc_psum.tile([N, 2 * H], f32, tag="prep")
            gf_ps = gf_full[:, 0:H]
            for jj in range(CJ):
                rhs = lfj_s[:, (jj * NB + b) * H:(jj * NB + b + 1) * H]
                nc.tensor.matmul(gf_ps[:], sig_s[:, jj * N:(jj + 1) * N], rhs,
                                 start=(jj == 0), stop=(jj == CJ - 1))
            gf_s = perb_p.tile([N, H], bf16, tag="gfs")
            nc.scalar.copy(gf_s[:], gf_ps[:])
            nc.scalar.dma_start(tbl[b * N:(b + 1) * N, H:2 * H], gf_s[:])

            # gather lf|gf rows for this batch's edges; first PB idxs are
            # r1 (-> out blocks [0,NGB)), next PB are r2.
            gout = gath_p.tile([128, 2 * NGB, 2 * H], bf16, tag="gout")
            nc.gpsimd.dma_gather(gout[:], tbl[b * N:(b + 1) * N, :],
                                 ridx_s[:, b * IWB:(b + 1) * IWB],
                                 2 * PB, 2 * PB, 2 * H, single_packet=False)
            gsum = gath_p.tile([128, NGB, 2 * H], bf16, tag="gsum")
            # the add waits on the gather; on DVE that wait would block the
            # next batch's relu stream, so only the final (tail-critical)
            # add runs on DVE — the rest use gpsimd's slack
            addeng = nc.vector if b == NB - 1 else nc.gpsimd
            addeng.tensor_tensor(out=gsum[:], in0=gout[:, 0:NGB, :],
                                 in1=gout[:, NGB:2 * NGB, :],
                                 op=mybir.AluOpType.add)
            nc.sync.dma_start(
                out_d[b * PB:(b + 1) * PB, :].rearrange(
                    "(p g) e -> p (g e)", p=128), gsum[:])

    nc.compile()
    return nc


def _host_prep(local_feats, binary_feats, sparse_idx, W_apair, W_binary,
               b_binary, W_att, b_att):
    """Shard + lay out inputs per core; returns (in_maps, unshard info)."""
    lf = np.asarray(local_feats, dtype=np.float32)
    bf = np.asarray(binary_feats, dtype=np.float32)
    si = np.asarray(sparse_idx)

    b_idx, i_idx, j_idx = si[:, 0], si[:, 1], si[:, 2]
    core = (b_idx // NB).astype(np.int64)
    lbat = (b_idx % NB).astype(np.int64)
    seg_rows = [[np.nonzero((core == k) & (lbat == b))[0]
                 for b in range(NB)] for k in range(NCORES)]
    maxc = max(len(r) for segs in seg_rows for r in segs)
    PB = max(128, ((maxc + 127) // 128) * 128)
    NGB = PB // 128
    _cache["PB"] = PB
    _cache["GPAD"] = NB * PB

    statics = _build_statics()
    watt = _build_watt_pad(np.asarray(W_att, dtype=np.float32))
    wbp = np.concatenate([np.asarray(b_binary, dtype=np.float32).reshape(1, H),
                          np.asarray(W_binary, dtype=np.float32)]).astype(BF16)
    batt = np.full((NCHUNK, 1), np.float32(np.asarray(b_att).reshape(-1)[0]),
                   dtype=np.float32)
    wa = np.asarray(W_apair, dtype=np.float32).astype(BF16)

    in_maps = []
    for k in range(NCORES):
        b0 = k * NB
        lf_k = lf[b0:b0 + NB].reshape(NB * N, H)
        lft_k = lf_k.T.astype(BF16)
        # column q of batch b = lf[b, CJ*(q%NCHUNK) + q//NCHUNK]
        qperm = (CJ * (np.arange(N) % NCHUNK) + np.arange(N) // NCHUNK)
        lftq_k = lf[b0:b0 + NB][:, qperm].reshape(NB * N, H).T.astype(BF16)
        # lfj tile: row c, col-block (g, b) holds lf[b, 5c+g]
        lfj_k = np.zeros((NCHUNK, CJ * NB * H), dtype=np.float32)
        for g in range(CJ):
            for c in range(NCHUNK):
                lfj_k[c, (g * NB) * H:(g * NB + NB) * H] = \
                    lf[b0:b0 + NB, CJ * c + g, :].reshape(-1)
        # [b, i, j, c] -> [b, c, (j, i)]
        bft_k = np.ascontiguousarray(
            bf[b0:b0 + NB].transpose(0, 3, 2, 1).reshape(NB, BIN, N * N)
        ).astype(BF16)
        # per-batch p-major gather index streams [r1 block | r2 block]
        ridx = np.zeros(NB * 2 * PB, dtype=np.int16)
        for b in range(NB):
            rows = seg_rows[k][b]
            r1 = np.zeros(PB, dtype=np.int16)
            r2 = np.zeros(PB, dtype=np.int16)
            r1[:len(rows)] = i_idx[rows].astype(np.int16)
            r2[:len(rows)] = j_idx[rows].astype(np.int16)
            # slot [p, g] <- idx[g*128+p]; out row p*NGB+g = seg position
            a1 = r1.reshape(128, NGB).T.reshape(-1)
            a2 = r2.reshape(128, NGB).T.reshape(-1)
            ridx[b * 2 * PB:b * 2 * PB + PB] = a1
            ridx[b * 2 * PB + PB:(b + 1) * 2 * PB] = a2
        blob = np.concatenate([wa, lft_k, lftq_k, watt], axis=1)
        in_maps.append({
            "bf_t": bft_k, "blob": blob, "batt": batt, "wbp": wbp,
            "lfj": lfj_k.astype(BF16), "statics": statics,
            "tbl_lf": lf_k.astype(BF16),
            "ridx": _wrap_idx(ridx),
        })
    return in_maps, seg_rows, PB


def _wrap_idx(idx):
    # int16 indices wrapped in 16 partitions (idx[k] at [k % 16, k // 16]),
    # replicated to all 8 gpsimd cores' partition groups.
    w = idx.reshape(-1, 16).T
    return np.ascontiguousarray(np.tile(w, (8, 1)))


def kernel(local_feats, binary_feats, sparse_idx, W_apair, W_binary,
           b_binary, W_att, b_att):
    in_maps, seg_rows, PB = _host_prep(
        local_feats, binary_feats, sparse_idx, W_apair, W_binary,
        b_binary, W_att, b_att)

    key = ("prog", NB * PB)
    if key not in _cache:
        _cache[key] = _build_program()
    nc = _cache[key]

    from concourse.bass_utils import run_bass_kernel_spmd
    trace = os.environ.get("KERNEL_TRACE", "0") == "1"
    res = run_bass_kernel_spmd(nc, in_maps, core_ids=list(range(NCORES)),
                               trace=trace)
    if trace and res.exec_time_ns is not None:
        print(f"HW exec time: {res.exec_time_ns} ns")
        print(f"mean exec time: {res.mean_exec_time_ns} ns")
        if res.instructions_and_trace is not None:
            print(f"trace: {res.instructions_and_trace[1]}")

    lp_full = np.empty((E, H), dtype=np.float32)
    gp_full = np.empty((E, H), dtype=np.float32)
    for k in range(NCORES):
        out = np.asarray(res.results[k]["out"]).astype(np.float32)
        for b in range(NB):
            rows = seg_rows[k][b]
            seg = out[b * PB:b * PB + len(rows)]
            lp_full[rows] = seg[:, 0:H]
            gp_full[rows] = seg[:, H:2 * H]
    return (lp_full, gp_full)
